# revision 31
# baseline (speedup 1.0000x reference)
"""Trainium2 Bass kernel for 2-layer GAT (nn_GAT_90460601188538).

Strategy: edges sorted by destination; destination nodes greedily packed
into 128-slot tiles; tiles split contiguously across 8 cores. Per
edge-chunk of 128, a one-hot selection matrix (iota == dst_slot) turns the
segmented softmax-sum and scatter-add into PE matmuls accumulating in
PSUM. Softmax runs without max subtraction (scores are O(1)) as
unnormalized sums plus one divide per node. Edge-source features are
fetched with dma_gather (int16 indices), so every table is split at one
global node threshold THR into two halves gathered separately; each tile's
chunks are partitioned into group A (src < THR) and group B. Pad edge
slots point at row 0 with dst slot 200, whose one-hot row is all zero, so
they contribute nothing. Layer-2 features are exchanged with an AllGather.
"""
import sys
sys.path.insert(0, '/opt/trn_rl_repo')
from contextlib import ExitStack

import numpy as np
import ml_dtypes

import concourse.bacc as bacc
import concourse.tile as tile
from concourse import bass, mybir, library_config
from concourse.bass_utils import run_bass_kernel_spmd

BF16 = ml_dtypes.bfloat16
F32 = np.float32

P = 128
NC = 8
CA = 12               # group-A chunks per tile (src < THR)
CB = 8                # group-B chunks per tile (src >= THR)
CPT = CA + CB
NEG_SLOPE = 0.2
NEG_BIG = -10000.0
ROW = 128             # table row elems (bf16) = 256B
H, C, F = 4, 64, 128
THR_CAP = 30720       # node split threshold cap (int16 table indexing)
B_STOP = 6            # debug: truncate phase-B body (1..6)


# ----------------------------------------------------------------- host prep

def _wrap_idx(flat):
    """[n] int -> dma_gather layout [128, n//16] int16 (16-wrap, replicated)."""
    n = flat.shape[0]
    out = np.zeros((P, n // 16), np.int16)
    cols = flat.reshape(n // 16, 16).T.astype(np.int16)   # [16, n//16]
    for rep in range(8):
        out[rep * 16:(rep + 1) * 16, :] = cols
    return out


def _prep(edge_index, n_nodes):
    src = np.concatenate([edge_index[0], np.arange(n_nodes, dtype=np.int64)])
    dst = np.concatenate([edge_index[1], np.arange(n_nodes, dtype=np.int64)])
    perm = np.argsort(dst, kind='stable')
    src_s = src[perm].astype(np.int64)
    dst_s = dst[perm].astype(np.int64)
    deg = np.bincount(dst_s, minlength=n_nodes)

    THR = min(THR_CAP, n_nodes)   # node split threshold (A: src < THR)
    capA, capB = CA * P, CB * P

    # greedy pack consecutive nodes: <=P nodes, <=capA A-edges, <=capB B-edges
    isB = (src_s >= THR)
    degA = np.bincount(dst_s[~isB], minlength=n_nodes)
    degB = deg - degA
    cumA = np.concatenate([[0], np.cumsum(degA)])
    cumB = np.concatenate([[0], np.cumsum(degB)])
    tiles = []
    n = 0
    while n < n_nodes:
        hiA = int(np.searchsorted(cumA, cumA[n] + capA, side='right')) - 1 - n
        hiB = int(np.searchsorted(cumB, cumB[n] + capB, side='right')) - 1 - n
        cnt = max(1, min(hiA, hiB, P, n_nodes - n))
        assert cumA[n + cnt] - cumA[n] <= capA
        assert cumB[n + cnt] - cumB[n] <= capB
        tiles.append((n, cnt))
        n += cnt
    T = len(tiles)
    NT = (T + 1 + NC - 1) // NC      # >=1 pad tile overall
    NCH = NT * CPT

    nrow1 = ((n_nodes + P - 1) // P) * P
    nrow2 = NC * NT * P

    # node -> (global slot row, tile)
    node_row = np.zeros(n_nodes, np.int64)
    nstart = np.zeros((NC, NT), np.int64)
    ncnt = np.zeros((NC, NT), np.int64)
    for gi, (n0, cnt) in enumerate(tiles):
        c, t = gi // NT, gi % NT
        nstart[c, t] = n0
        ncnt[c, t] = cnt
        node_row[n0:n0 + cnt] = gi * P + np.arange(cnt)
    RB = int(node_row[THR]) if THR < n_nodes else max(0, nrow2 - P)
    assert RB < 32768 and nrow2 - RB <= 32768, (RB, nrow2)
    assert nrow1 - THR <= 32768, (THR, nrow1)

    # per-core streams
    idxA1 = np.zeros((NC, P, NT * CA * 8), np.int16)
    idxB1 = np.zeros((NC, P, NT * CB * 8), np.int16)
    idxA2 = np.zeros((NC, P, NT * CA * 8), np.int16)
    idxB2 = np.zeros((NC, P, NT * CB * 8), np.int16)
    dstloc = np.full((NC, P, NCH), BF16(200.0), BF16)
    adoffs = np.zeros((NC, P, NT), np.int32)      # slot node ids (a_d tile)
    t2bias = np.full((NC, P, NT * 2), NEG_BIG, F32)

    # edge ranges per tile
    epos = 0
    edge_of_tile = []
    for (n0, cnt) in tiles:
        e0 = epos
        ecnt = int(deg[n0:n0 + cnt].sum())
        edge_of_tile.append((e0, ecnt))
        epos += ecnt

    for gi, (n0, cnt) in enumerate(tiles):
        c, t = gi // NT, gi % NT
        e0, ecnt = edge_of_tile[gi]
        es = src_s[e0:e0 + ecnt]
        ed = dst_s[e0:e0 + ecnt]
        sl = (ed - n0).astype(np.int64)
        selB = es >= THR
        esA, slA = es[~selB], sl[~selB]
        esB, slB = es[selB], sl[selB]
        fa = np.zeros(capA, np.int64)
        fa[:len(esA)] = esA
        fb = np.zeros(capB, np.int64)
        fb[:len(esB)] = esB - THR
        idxA1[c, :, t * CA * 8:(t + 1) * CA * 8] = _wrap_idx(fa)
        idxB1[c, :, t * CB * 8:(t + 1) * CB * 8] = _wrap_idx(fb)
        fa2 = np.zeros(capA, np.int64)
        fa2[:len(esA)] = node_row[esA]
        fb2 = np.zeros(capB, np.int64)
        fb2[:len(esB)] = node_row[esB] - RB
        idxA2[c, :, t * CA * 8:(t + 1) * CA * 8] = _wrap_idx(fa2)
        idxB2[c, :, t * CB * 8:(t + 1) * CB * 8] = _wrap_idx(fb2)
        # dstloc: chunks 0..CA-1 = A slots, CA..CPT-1 = B slots
        dl = np.full((CPT, P), 200.0, np.float64)
        ia = np.arange(len(esA))
        dl[ia // P, ia % P] = slA
        ib = np.arange(len(esB))
        dl[CA + ib // P, ib % P] = slB
        dstloc[c, :, t * CPT:(t + 1) * CPT] = dl.T.astype(BF16)
        ad = np.full(P, n_nodes, np.int64)    # pad slots -> dummy row
        ad[:cnt] = n0 + np.arange(cnt)
        adoffs[c, :, t] = ad
        t2bias[c, :, 2 * t:2 * t + 2] = 0.0

    meta = dict(idxA1=idxA1, idxB1=idxB1, idxA2=idxA2, idxB2=idxB2,
                dstloc=dstloc, adoffs=adoffs, t2bias=t2bias)
    shapes = dict(T=T, NT=NT, NCH=NCH, nrow1=nrow1, nrow2=nrow2,
                  THR=THR, RB=RB)
    return meta, shapes, nstart, ncnt


def _host_tables(x, W1, att_src1, att_dst1, W2, att_src2, att_dst2, shapes):
    n_nodes = x.shape[0]
    nrow1 = shapes['nrow1']

    t1x = np.zeros((nrow1, 2 * ROW), BF16)
    t1x[:n_nodes, :F] = x.astype(BF16)

    xT = np.zeros((P, nrow1), BF16)
    xT[:, :n_nodes] = x.astype(BF16).T

    W1r = W1.reshape(F, H, C)
    wsd = np.zeros((P, 8), BF16)
    wsd[:, 0:4] = np.einsum('fhc,hc->fh', W1r, att_src1).astype(BF16)
    wsd[:, 4:8] = np.einsum('fhc,hc->fh', W1r, att_dst1).astype(BF16)

    w1 = W1.astype(BF16)                                   # [128, 256]
    ws2 = (W2 @ att_src2[0])[:, None]
    wd2 = (W2 @ att_dst2[0])[:, None]
    w2e = np.concatenate([W2, ws2, wd2], axis=1).astype(BF16)  # [256, 42]
    w2e_packed = np.concatenate([w2e[0:P], w2e[P:2 * P]], axis=1)  # [128, 84]

    iota_k = np.tile(np.arange(P, dtype=BF16), CPT)[None, :].repeat(P, 0)
    ident = np.eye(P, dtype=BF16)
    return dict(t1x=t1x, xT=xT, wsd=wsd, w1=w1, w2e=w2e_packed,
                iota_k=np.ascontiguousarray(iota_k), ident=ident)


# ------------------------------------------------------------- device program

def _build(shapes, n_nodes, debug=False, phases='ABCD'):
    B_STOP = globals()['B_STOP']
    NT, NCH = shapes['NT'], shapes['NCH']
    nrow1, nrow2 = shapes['nrow1'], shapes['nrow2']
    THR, RB = shapes['THR'], shapes['RB']
    NAT = nrow1
    bf = mybir.dt.bfloat16
    f32 = mybir.dt.float32
    i32 = mybir.dt.int32
    i16 = mybir.dt.int16
    AG_CHUNKS = 4
    assert NT % AG_CHUNKS == 0 or True

    nc = bacc.Bacc('TRN2', target_bir_lowering=False, debug=False,
                   num_devices=NC)

    def inp(name, shape, dt):
        return nc.dram_tensor(name, list(shape), dt, kind='ExternalInput').ap()

    t1x = inp('t1x', (nrow1, 2 * ROW), bf)
    xT = inp('xT', (P, nrow1), bf)
    wsd = inp('wsd', (P, 8), bf)
    w1 = inp('w1', (P, H * C), bf)
    w2e = inp('w2e', (P, 2 * 42), bf)
    iota_k = inp('iota_k', (P, CPT * P), bf)
    ident = inp('ident', (P, P), bf)
    b1bc = inp('b1bc', (P, H * C), f32)
    b2bc = inp('b2bc', (P, 40), f32)
    t2bias = inp('t2bias', (P, NT * 2), f32)
    m_idxA1 = inp('idxA1', (P, NT * CA * 8), i16)
    m_idxB1 = inp('idxB1', (P, NT * CB * 8), i16)
    m_idxA2 = inp('idxA2', (P, NT * CA * 8), i16)
    m_idxB2 = inp('idxB2', (P, NT * CB * 8), i16)
    m_dstloc = inp('dstloc', (P, NCH), bf)
    m_adoffs = inp('adoffs', (P, NT), i32)

    out_d = nc.dram_tensor('out', [NT * P, 40], f32, kind='ExternalOutput').ap()

    adtab = nc.dram_tensor('adtab', [NAT, 4], bf).ap()     # [a_d(4)]

    dbg = {}

    with tile.TileContext(nc) as tc, ExitStack() as ctx:
        nc.gpsimd.load_library(library_config.mlp)
        dram = ctx.enter_context(tc.tile_pool(name='dram', bufs=1, space='DRAM'))
        t2_local = dram.tile([NT * P, ROW], bf)
        t2_full = dram.tile([nrow2, ROW], bf, addr_space='Shared')

        consts = ctx.enter_context(tc.tile_pool(name='consts', bufs=1))
        meta = ctx.enter_context(tc.tile_pool(name='meta', bufs=1))

        s_wsd = consts.tile([P, 8], bf)
        nc.sync.dma_start(out=s_wsd, in_=wsd)
        s_w1 = consts.tile([P, H * C], bf)
        nc.sync.dma_start(out=s_w1, in_=w1)
        s_w2e = consts.tile([P, 2 * 42], bf)
        nc.sync.dma_start(out=s_w2e, in_=w2e)
        s_iota = consts.tile([P, CPT * P], bf)
        nc.sync.dma_start(out=s_iota, in_=iota_k)
        s_ident = consts.tile([P, P], bf)
        nc.sync.dma_start(out=s_ident, in_=ident)
        s_b1 = consts.tile([P, H * C], f32)
        nc.sync.dma_start(out=s_b1, in_=b1bc)
        s_b2 = consts.tile([P, 40], f32)
        nc.sync.dma_start(out=s_b2, in_=b2bc)
        s_t2bias = consts.tile([P, NT * 2], f32)
        nc.sync.dma_start(out=s_t2bias, in_=t2bias)

        s_idxA1 = meta.tile([P, NT * CA * 8], i16)
        nc.sync.dma_start(out=s_idxA1, in_=m_idxA1)
        s_idxB1 = meta.tile([P, NT * CB * 8], i16)
        nc.sync.dma_start(out=s_idxB1, in_=m_idxB1)
        s_idxA2 = meta.tile([P, NT * CA * 8], i16)
        nc.sync.dma_start(out=s_idxA2, in_=m_idxA2)
        s_idxB2 = meta.tile([P, NT * CB * 8], i16)
        nc.sync.dma_start(out=s_idxB2, in_=m_idxB2)
        s_dstloc = meta.tile([P, NCH], bf)
        nc.sync.dma_start(out=s_dstloc, in_=m_dstloc)
        s_adoffs = meta.tile([P, NT], i32)
        nc.sync.dma_start(out=s_adoffs, in_=m_adoffs)

        # ---------------- phase A: a_s (astab) and a_d (adtab) tables ------
        GA = 8
        n_a_tiles = nrow1 // P
        with tc.tile_pool(name='pa', bufs=2) as pa, \
             tc.tile_pool(name='pa_ps', bufs=2, space='PSUM') as pa_ps:
            for t0 in range(0, n_a_tiles if 'A' in phases else 0, GA):
                g = min(GA, n_a_tiles - t0)
                xt = pa.tile([P, GA * P], bf, tag='xt')
                nc.sync.dma_start(out=xt[:, :g * P],
                                  in_=xT[:, t0 * P:(t0 + g) * P])
                ps = pa_ps.tile([P, GA * 8], f32, tag='ps')
                for j in range(g):
                    nc.tensor.matmul(out=ps[:, j * 8:(j + 1) * 8],
                                     lhsT=xt[:, j * P:(j + 1) * P],
                                     rhs=s_wsd, start=True, stop=True)
                sa = pa.tile([P, GA * 4], bf, tag='sa')
                nc.vector.tensor_copy(
                    out=sa[:, :g * 4].rearrange('p (j e) -> p j e', e=4),
                    in_=ps[:, :g * 8].rearrange('p (j e) -> p j e', e=8)[:, :, 0:4])
                sd = pa.tile([P, GA * 4], bf, tag='sd')
                nc.vector.tensor_copy(
                    out=sd[:, :g * 4].rearrange('p (j e) -> p j e', e=4),
                    in_=ps[:, :g * 8].rearrange('p (j e) -> p j e', e=8)[:, :, 4:8])
                as_ap = bass.AP(tensor=t1x.tensor,
                                offset=t0 * P * 2 * ROW + F,
                                ap=[[2 * ROW, P], [P * 2 * ROW, g], [1, 4]])
                nc.sync.dma_start(
                    out=as_ap,
                    in_=sa[:, :g * 4].rearrange('p (j e) -> p j e', e=4))
                ad_ap = bass.AP(tensor=adtab.tensor,
                                offset=t0 * P * 4,
                                ap=[[4, P], [P * 4, g], [1, 4]])
                nc.sync.dma_start(
                    out=ad_ap,
                    in_=sd[:, :g * 4].rearrange('p (j e) -> p j e', e=4))

        if 'Z' in phases:   # minimal: write zeros to out
            with tc.tile_pool(name='pz', bufs=1) as pz:
                zt = pz.tile([P, 40], f32)
                nc.vector.memset(zt, 0.0)
                for t in range(NT):
                    nc.sync.dma_start(out=out_d[t * P:(t + 1) * P, :], in_=zt)

        # ---------------- phase B: layer-1 edges + tile epilogue -----------
        with tc.tile_pool(name='pb', bufs=2) as pb, \
             tc.tile_pool(name='pb_rhs', bufs=2) as pb_rhs, \
             tc.tile_pool(name='pb_ep', bufs=2) as pb_ep, \
             tc.tile_pool(name='ps_acc', bufs=1, space='PSUM') as ps_acc, \
             tc.tile_pool(name='ps_sm', bufs=2, space='PSUM') as ps_sm, \
             tc.tile_pool(name='ps_ep', bufs=1, space='PSUM') as ps_ep:
            for t in range(NT if 'B' in phases else 0):
                c0 = t * CPT
                gx = pb.tile([P, CPT, 2 * ROW], bf, tag='gx')
                nc.gpsimd.dma_gather(gx[:, 0:CA, :], t1x,
                                     s_idxA1[:, t * CA * 8:(t + 1) * CA * 8],
                                     CA * P, CA * P, 2 * ROW,
                                     single_packet=False)
                nc.gpsimd.dma_gather(gx[:, CA:CPT, :], t1x[THR:nrow1, :],
                                     s_idxB1[:, t * CB * 8:(t + 1) * CB * 8],
                                     CB * P, CB * P, 2 * ROW,
                                     single_packet=False)
                adt = pb.tile([P, 4], bf, tag='adt')
                nc.gpsimd.indirect_dma_start(
                    out=adt, out_offset=None, in_=adtab,
                    in_offset=bass.IndirectOffsetOnAxis(
                        ap=s_adoffs[:, t:t + 1], axis=0))
                if B_STOP < 2:
                    continue
                # one-hot S01 for the whole tile
                s01 = pb.tile([P, CPT * P], bf, tag='s01')
                nc.vector.tensor_tensor(
                    out=s01.rearrange('p (k j) -> p k j', j=P),
                    in0=s_iota.rearrange('p (k j) -> p k j', j=P),
                    in1=s_dstloc[:, c0:c0 + CPT].to_broadcast([P, CPT, P]),
                    op=mybir.AluOpType.is_equal)
                # per-chunk: transpose S01 -> expand a_d -> scores
                sstage = pb.tile([P, CPT * 4], bf, tag='sstage')
                pss = [ps_acc.tile([P, P], f32, tag=f'ph{h}',
                                   name=f'ph{h}')
                       for h in range(H)]
                den_ps = ps_acc.tile([P, 8], f32, tag='den')
                w = pb.tile([P, CPT * 4], bf, tag='w')
                if B_STOP < 3:
                    continue
                for j in range(CPT):
                    psT = ps_sm.tile([P, P], bf, tag='psT', name='psT')
                    nc.tensor.transpose(out=psT, in_=s01[:, j * P:(j + 1) * P],
                                        identity=s_ident)
                    s01T = pb_rhs.tile([P, P], bf, tag='s01T')
                    nc.vector.tensor_copy(out=s01T, in_=psT)
                    adp = ps_sm.tile([P, 4], f32, tag='psT', name='adp')
                    nc.tensor.matmul(out=adp, lhsT=s01T, rhs=adt,
                                     start=True, stop=True)
                    nc.vector.tensor_tensor(
                        out=sstage[:, j * 4:(j + 1) * 4],
                        in0=gx[:, j, F:F + 4], in1=adp,
                        op=mybir.AluOpType.add)
                if B_STOP < 4:
                    continue
                st = pb.tile([P, CPT * 4], bf, tag='st')
                nc.vector.tensor_scalar(out=st, in0=sstage, scalar1=NEG_SLOPE,
                                        scalar2=None, op0=mybir.AluOpType.mult)
                sl = pb.tile([P, CPT * 4], bf, tag='sl')
                nc.vector.tensor_tensor(out=sl, in0=st, in1=sstage,
                                        op=mybir.AluOpType.max)
                nc.scalar.activation(w, sl, mybir.ActivationFunctionType.Exp)
                # rhs_h = w_h * x ; accumulate
                rhs = []
                w3 = w.rearrange('p (k e) -> p k e', e=4)
                for h in range(H):
                    r = pb_rhs.tile([P, CPT * P], bf, tag=f'rhs{h}')
                    nc.vector.tensor_tensor(
                        out=r.rearrange('p (k j) -> p k j', j=P),
                        in0=gx[:, :, 0:F],
                        in1=w3[:, :, h:h + 1].to_broadcast([P, CPT, P]),
                        op=mybir.AluOpType.mult)
                    rhs.append(r)
                if B_STOP < 5:
                    continue
                for j in range(CPT):
                    for h in range(H):
                        nc.tensor.matmul(
                            out=pss[h][:, 0:128],
                            lhsT=s01[:, j * P:(j + 1) * P],
                            rhs=rhs[h][:, j * P:(j + 1) * P],
                            start=(j == 0), stop=(j == CPT - 1))
                    nc.tensor.matmul(
                        out=den_ps[:, 0:4],
                        lhsT=s01[:, j * P:(j + 1) * P],
                        rhs=w[:, j * 4:(j + 1) * 4],
                        start=(j == 0), stop=(j == CPT - 1))
                # epilogue
                if B_STOP < 6:
                    continue
                den = pb_ep.tile([P, 4], f32, tag='den4')
                nc.vector.tensor_scalar(out=den, in0=den_ps[:, 0:4],
                                        scalar1=1e-20, scalar2=None,
                                        op0=mybir.AluOpType.max)
                rec = pb_ep.tile([P, 4], f32, tag='rec')
                nc.vector.reciprocal(out=rec, in_=den)
                out1 = ps_ep.tile([P, H * C], f32, tag='ep')
                for h in range(H):
                    an = pb_ep.tile([P, P], bf, tag=f'an{h}')
                    nc.scalar.activation(an, pss[h][:, 0:128],
                                         mybir.ActivationFunctionType.Copy,
                                         scale=rec[:, h:h + 1])
                    psT2 = ps_sm.tile([P, P], bf, tag='psT', name='psT2')
                    nc.tensor.transpose(out=psT2, in_=an, identity=s_ident)
                    anT = pb_ep.tile([P, P], bf, tag=f'anT{h}')
                    nc.vector.tensor_copy(out=anT, in_=psT2)
                    nc.tensor.matmul(out=out1[:, h * C:(h + 1) * C],
                                     lhsT=anT,
                                     rhs=s_w1[:, h * C:(h + 1) * C],
                                     start=True, stop=True)
                zb = pb_ep.tile([P, H * C], f32, tag='zb')
                nc.vector.tensor_tensor(out=zb, in0=out1, in1=s_b1,
                                        op=mybir.AluOpType.add)
                zm = pb_ep.tile([P, H * C], f32, tag='zm')
                nc.vector.tensor_scalar(out=zm, in0=zb, scalar1=0.0,
                                        scalar2=None, op0=mybir.AluOpType.min)
                ze = pb_ep.tile([P, H * C], f32, tag='ze')
                nc.scalar.activation(ze, zm, mybir.ActivationFunctionType.Exp)
                zr = pb_ep.tile([P, H * C], f32, tag='zr')
                nc.vector.tensor_scalar(out=zr, in0=zb, scalar1=0.0,
                                        scalar2=None, op0=mybir.AluOpType.max)
                zs = pb_ep.tile([P, H * C], f32, tag='zs')
                nc.vector.tensor_tensor(out=zs, in0=ze, in1=zr,
                                        op=mybir.AluOpType.add)
                hb = pb_ep.tile([P, H * C], bf, tag='hb')
                nc.vector.tensor_scalar(out=hb, in0=zs, scalar1=-1.0,
                                        scalar2=None, op0=mybir.AluOpType.add)
                xw2 = ps_ep.tile([P, 42], f32, tag='ep', name='xw2')
                for kk in range(2):
                    psT3 = ps_sm.tile([P, P], bf, tag='psT', name='psT3')
                    nc.tensor.transpose(out=psT3, in_=hb[:, kk * P:(kk + 1) * P],
                                        identity=s_ident)
                    hT = pb_ep.tile([P, P], bf, tag=f'hT{kk}')
                    nc.vector.tensor_copy(out=hT, in_=psT3)
                    nc.tensor.matmul(out=xw2, lhsT=hT,
                                     rhs=s_w2e[:, kk * 42:(kk + 1) * 42],
                                     start=(kk == 0), stop=(kk == 1))
                t2r = pb_ep.tile([P, ROW], bf, tag='t2r')
                nc.vector.memset(t2r[:, 42:ROW], 0.0)
                nc.vector.tensor_copy(out=t2r[:, 0:40], in_=xw2[:, 0:40])
                nc.vector.tensor_tensor(out=t2r[:, 40:42], in0=xw2[:, 40:42],
                                        in1=s_t2bias[:, 2 * t:2 * t + 2],
                                        op=mybir.AluOpType.add)
                nc.sync.dma_start(out=t2_local[t * P:(t + 1) * P, :], in_=t2r)

        # ---------------- phase C: allgather (chunked) ----------------
        CHK = NT // AG_CHUNKS if NT % AG_CHUNKS == 0 else NT
        nch = NT // CHK
        for a in range(nch if 'C' in phases else 0):
            nc.gpsimd.collective_compute(
                'AllGather', mybir.AluOpType.bypass,
                ins=[t2_local[a * CHK * P:(a + 1) * CHK * P, :]],
                outs=[t2_full.rearrange('(c r) e -> c r e', c=NC)
                      [:, a * CHK * P:(a + 1) * CHK * P, :]],
                replica_groups=[list(range(NC))])

        # ---------------- phase D: layer-2 edges + log_softmax ------------
        with tc.tile_pool(name='pd', bufs=2) as pd, \
             tc.tile_pool(name='pd_rhs', bufs=2) as pd_rhs, \
             tc.tile_pool(name='pd_ep', bufs=2) as pd_ep, \
             tc.tile_pool(name='ps2', bufs=2, space='PSUM') as ps2, \
             tc.tile_pool(name='ps2_sm', bufs=2, space='PSUM') as ps2_sm:
            for t in range(NT if 'D' in phases else 0):
                c0 = t * CPT
                g2 = pd.tile([P, CPT, ROW], bf, tag='g2')
                nc.gpsimd.dma_gather(g2[:, 0:CA, :], t2_full[:, :],
                                     s_idxA2[:, t * CA * 8:(t + 1) * CA * 8],
                                     CA * P, CA * P, ROW, single_packet=False)
                nc.gpsimd.dma_gather(g2[:, CA:CPT, :], t2_full[RB:nrow2, :],
                                     s_idxB2[:, t * CB * 8:(t + 1) * CB * 8],
                                     CB * P, CB * P, ROW, single_packet=False)
                ad2 = pd.tile([P, 2], bf, tag='ad2')
                nc.sync.dma_start(out=ad2,
                                  in_=t2_local[t * P:(t + 1) * P, 40:42])
                s01 = pd.tile([P, CPT * P], bf, tag='s01b')
                nc.vector.tensor_tensor(
                    out=s01.rearrange('p (k j) -> p k j', j=P),
                    in0=s_iota.rearrange('p (k j) -> p k j', j=P),
                    in1=s_dstloc[:, c0:c0 + CPT].to_broadcast([P, CPT, P]),
                    op=mybir.AluOpType.is_equal)
                sstage = pd.tile([P, CPT], bf, tag='sst2')
                acc = ps2.tile([P, 40], f32, tag='acc2')
                den_ps = ps2.tile([P, 8], f32, tag='den2p')
                for j in range(CPT):
                    psT = ps2_sm.tile([P, P], bf, tag='psT2', name='psTD')
                    nc.tensor.transpose(out=psT, in_=s01[:, j * P:(j + 1) * P],
                                        identity=s_ident)
                    s01T = pd_rhs.tile([P, P], bf, tag='s01T2')
                    nc.vector.tensor_copy(out=s01T, in_=psT)
                    adp = ps2_sm.tile([P, 1], f32, tag='psT2', name='adpD')
                    nc.tensor.matmul(out=adp, lhsT=s01T, rhs=ad2[:, 1:2],
                                     start=True, stop=True)
                    nc.vector.tensor_tensor(
                        out=sstage[:, j:j + 1],
                        in0=g2[:, j, 40:41], in1=adp,
                        op=mybir.AluOpType.add)
                st = pd.tile([P, CPT], bf, tag='st2')
                nc.vector.tensor_scalar(out=st, in0=sstage, scalar1=NEG_SLOPE,
                                        scalar2=None, op0=mybir.AluOpType.mult)
                sl = pd.tile([P, CPT], bf, tag='sl2')
                nc.vector.tensor_tensor(out=sl, in0=st, in1=sstage,
                                        op=mybir.AluOpType.max)
                w = pd.tile([P, CPT], bf, tag='w2')
                nc.scalar.activation(w, sl, mybir.ActivationFunctionType.Exp)
                r = pd_rhs.tile([P, CPT * 40], bf, tag='rhs2')
                nc.vector.tensor_tensor(
                    out=r.rearrange('p (k j) -> p k j', j=40),
                    in0=g2[:, :, 0:40],
                    in1=w.rearrange('p (k o) -> p k o', o=1)
                    .to_broadcast([P, CPT, 40]),
                    op=mybir.AluOpType.mult)
                for j in range(CPT):
                    nc.tensor.matmul(out=acc[:, :],
                                     lhsT=s01[:, j * P:(j + 1) * P],
                                     rhs=r[:, j * 40:(j + 1) * 40],
                                     start=(j == 0), stop=(j == CPT - 1))
                    nc.tensor.matmul(out=den_ps[:, 0:1],
                                     lhsT=s01[:, j * P:(j + 1) * P],
                                     rhs=w[:, j:j + 1],
                                     start=(j == 0), stop=(j == CPT - 1))
                den = pd_ep.tile([P, 1], f32, tag='den2')
                nc.vector.tensor_scalar(out=den, in0=den_ps[:, 0:1],
                                        scalar1=1e-20, scalar2=None,
                                        op0=mybir.AluOpType.max)
                rec = pd_ep.tile([P, 1], f32, tag='rec2')
                nc.vector.reciprocal(out=rec, in_=den)
                o = pd_ep.tile([P, 40], f32, tag='o')
                nc.scalar.activation(o, acc[:, 0:40],
                                     mybir.ActivationFunctionType.Copy,
                                     scale=rec)
                ob = pd_ep.tile([P, 40], f32, tag='ob')
                nc.vector.tensor_tensor(out=ob, in0=o, in1=s_b2,
                                        op=mybir.AluOpType.add)
                mx = pd_ep.tile([P, 1], f32, tag='mx')
                nc.vector.tensor_reduce(out=mx, in_=ob,
                                        axis=mybir.AxisListType.X,
                                        op=mybir.AluOpType.max)
                om = pd_ep.tile([P, 40], f32, tag='om')
                nc.vector.tensor_scalar(out=om, in0=ob, scalar1=mx,
                                        scalar2=None,
                                        op0=mybir.AluOpType.subtract)
                ex = pd_ep.tile([P, 40], f32, tag='ex')
                sm = pd_ep.tile([P, 1], f32, tag='sm')
                nc.scalar.activation(ex, om, mybir.ActivationFunctionType.Exp,
                                     accum_out=sm)
                lg = pd_ep.tile([P, 1], f32, tag='lg')
                nc.scalar.activation(lg, sm, mybir.ActivationFunctionType.Ln)
                fin = pd_ep.tile([P, 40], f32, tag='fin')
                nc.vector.tensor_scalar(out=fin, in0=om, scalar1=lg,
                                        scalar2=None,
                                        op0=mybir.AluOpType.subtract)
                nc.sync.dma_start(out=out_d[t * P:(t + 1) * P, :], in_=fin)

    nc.compile()
    return nc


# ----------------------------------------------------------------- entry

_CACHE = {}


def prepare(x, edge_index, W1, att_src1, att_dst1, b1, W2, att_src2, att_dst2,
            b2, build=True, debug=False):
    x = np.asarray(x, F32)
    edge_index = np.asarray(edge_index)
    n_nodes = x.shape[0]

    meta, shapes, nstart, ncnt = _prep(edge_index, n_nodes)
    tables = _host_tables(x, np.asarray(W1, F32), np.asarray(att_src1, F32),
                          np.asarray(att_dst1, F32), np.asarray(W2, F32),
                          np.asarray(att_src2, F32), np.asarray(att_dst2, F32),
                          shapes)
    nc = None
    if build:
        key = (shapes['NT'], shapes['THR'], shapes['RB'], n_nodes, debug)
        if key not in _CACHE:
            _CACHE[key] = _build(shapes, n_nodes, debug=debug)
        nc = _CACHE[key]

    b1bc = np.broadcast_to(np.asarray(b1, F32), (P, H * C)).copy()
    b2bc = np.broadcast_to(np.asarray(b2, F32), (P, 40)).copy()

    in_maps = []
    for c in range(NC):
        in_maps.append(dict(
            t1x=tables['t1x'], xT=tables['xT'], wsd=tables['wsd'],
            w1=tables['w1'], w2e=tables['w2e'], iota_k=tables['iota_k'],
            ident=tables['ident'], b1bc=b1bc, b2bc=b2bc,
            t2bias=meta['t2bias'][c],
            idxA1=meta['idxA1'][c], idxB1=meta['idxB1'][c],
            idxA2=meta['idxA2'][c], idxB2=meta['idxB2'][c],
            dstloc=meta['dstloc'][c], adoffs=meta['adoffs'][c],
        ))
    return dict(nc=nc, in_maps=in_maps, shapes=shapes, nstart=nstart,
                ncnt=ncnt, n_nodes=n_nodes)


def assemble(ctx_run, outs):
    NT = ctx_run['shapes']['NT']
    nstart, ncnt = ctx_run['nstart'], ctx_run['ncnt']
    out = np.zeros((ctx_run['n_nodes'], 40), F32)
    for c in range(NC):
        oc = outs[c]['out']
        for t in range(NT):
            cnt = int(ncnt[c, t])
            if cnt == 0:
                continue
            n0 = int(nstart[c, t])
            out[n0:n0 + cnt] = oc[t * P:t * P + cnt]
    return out


def kernel(x, edge_index, W1, att_src1, att_dst1, b1, W2, att_src2, att_dst2, b2):
    ctx_run = prepare(x, edge_index, W1, att_src1, att_dst1, b1,
                      W2, att_src2, att_dst2, b2)
    res = run_bass_kernel_spmd(ctx_run['nc'], ctx_run['in_maps'],
                               list(range(NC)))
    return assemble(ctx_run, res.results)


# revision 32
# speedup vs baseline: 1.4610x; 1.4610x over previous
"""Trainium2 Bass kernel for 2-layer GAT (nn_GAT_90460601188538).

Strategy: edges sorted by destination; destination nodes greedily packed
into 128-slot tiles; tiles split contiguously across 8 cores. Per
edge-chunk of 128, a one-hot selection matrix (iota == dst_slot) turns the
segmented softmax-sum and scatter-add into PE matmuls accumulating in
PSUM. Softmax runs without max subtraction (scores are O(1)) as
unnormalized sums plus one divide per node. Edge-source features are
fetched with dma_gather (int16 indices), so every table is split at one
global node threshold THR into two halves gathered separately; each tile's
chunks are partitioned into group A (src < THR) and group B. Pad edge
slots point at row 0 with dst slot 200, whose one-hot row is all zero, so
they contribute nothing. Layer-2 features are exchanged with an AllGather.
"""
import sys
sys.path.insert(0, '/opt/trn_rl_repo')
from contextlib import ExitStack

import numpy as np
import ml_dtypes

import concourse.bacc as bacc
import concourse.tile as tile
from concourse import bass, mybir, library_config
from concourse.bass_utils import run_bass_kernel_spmd

BF16 = ml_dtypes.bfloat16
F32 = np.float32

P = 128
NC = 8
CA = 12               # group-A chunks per tile (src < THR)
CB = 8                # group-B chunks per tile (src >= THR)
CPT = CA + CB
NEG_SLOPE = 0.2
NEG_BIG = -10000.0
ROW = 128             # table row elems (bf16) = 256B
H, C, F = 4, 64, 128
THR_CAP = 30720       # node split threshold cap (int16 table indexing)
B_STOP = 6            # debug: truncate phase-B body (1..6)


# ----------------------------------------------------------------- host prep

def _wrap_idx(flat):
    """[n] int -> dma_gather layout [128, n//16] int16 (16-wrap, replicated)."""
    n = flat.shape[0]
    out = np.zeros((P, n // 16), np.int16)
    cols = flat.reshape(n // 16, 16).T.astype(np.int16)   # [16, n//16]
    for rep in range(8):
        out[rep * 16:(rep + 1) * 16, :] = cols
    return out


def _prep(edge_index, n_nodes):
    src = np.concatenate([edge_index[0], np.arange(n_nodes, dtype=np.int64)])
    dst = np.concatenate([edge_index[1], np.arange(n_nodes, dtype=np.int64)])
    perm = np.argsort(dst, kind='stable')
    src_s = src[perm].astype(np.int64)
    dst_s = dst[perm].astype(np.int64)
    deg = np.bincount(dst_s, minlength=n_nodes)

    THR = min(THR_CAP, n_nodes)   # node split threshold (A: src < THR)
    capA, capB = CA * P, CB * P

    # greedy pack consecutive nodes: <=P nodes, <=capA A-edges, <=capB B-edges
    isB = (src_s >= THR)
    degA = np.bincount(dst_s[~isB], minlength=n_nodes)
    degB = deg - degA
    cumA = np.concatenate([[0], np.cumsum(degA)])
    cumB = np.concatenate([[0], np.cumsum(degB)])
    tiles = []
    n = 0
    while n < n_nodes:
        hiA = int(np.searchsorted(cumA, cumA[n] + capA, side='right')) - 1 - n
        hiB = int(np.searchsorted(cumB, cumB[n] + capB, side='right')) - 1 - n
        cnt = max(1, min(hiA, hiB, P, n_nodes - n))
        assert cumA[n + cnt] - cumA[n] <= capA
        assert cumB[n + cnt] - cumB[n] <= capB
        tiles.append((n, cnt))
        n += cnt
    T = len(tiles)
    NT = (T + 1 + NC - 1) // NC      # >=1 pad tile overall
    NCH = NT * CPT

    nrow1 = ((n_nodes + P - 1) // P) * P
    nrow2 = NC * NT * P

    # node -> (global slot row, tile)
    node_row = np.zeros(n_nodes, np.int64)
    nstart = np.zeros((NC, NT), np.int64)
    ncnt = np.zeros((NC, NT), np.int64)
    for gi, (n0, cnt) in enumerate(tiles):
        c, t = gi // NT, gi % NT
        nstart[c, t] = n0
        ncnt[c, t] = cnt
        node_row[n0:n0 + cnt] = gi * P + np.arange(cnt)
    RB = int(node_row[THR]) if THR < n_nodes else max(0, nrow2 - P)
    assert RB < 32768 and nrow2 - RB <= 32768, (RB, nrow2)
    assert nrow1 - THR <= 32768, (THR, nrow1)

    # per-core streams
    idxA1 = np.zeros((NC, P, NT * CA * 8), np.int16)
    idxB1 = np.zeros((NC, P, NT * CB * 8), np.int16)
    idxA2 = np.zeros((NC, P, NT * CA * 8), np.int16)
    idxB2 = np.zeros((NC, P, NT * CB * 8), np.int16)
    dstloc = np.full((NC, P, NCH), BF16(200.0), BF16)
    adoffs = np.zeros((NC, P, NT), np.int32)      # slot node ids (a_d tile)
    t2bias = np.full((NC, P, NT * 2), NEG_BIG, F32)

    # edge ranges per tile
    epos = 0
    edge_of_tile = []
    for (n0, cnt) in tiles:
        e0 = epos
        ecnt = int(deg[n0:n0 + cnt].sum())
        edge_of_tile.append((e0, ecnt))
        epos += ecnt

    for gi, (n0, cnt) in enumerate(tiles):
        c, t = gi // NT, gi % NT
        e0, ecnt = edge_of_tile[gi]
        es = src_s[e0:e0 + ecnt]
        ed = dst_s[e0:e0 + ecnt]
        sl = (ed - n0).astype(np.int64)
        selB = es >= THR
        esA, slA = es[~selB], sl[~selB]
        esB, slB = es[selB], sl[selB]
        fa = np.zeros(capA, np.int64)
        fa[:len(esA)] = esA
        fb = np.zeros(capB, np.int64)
        fb[:len(esB)] = esB - THR
        idxA1[c, :, t * CA * 8:(t + 1) * CA * 8] = _wrap_idx(fa)
        idxB1[c, :, t * CB * 8:(t + 1) * CB * 8] = _wrap_idx(fb)
        fa2 = np.zeros(capA, np.int64)
        fa2[:len(esA)] = node_row[esA]
        fb2 = np.zeros(capB, np.int64)
        fb2[:len(esB)] = node_row[esB] - RB
        idxA2[c, :, t * CA * 8:(t + 1) * CA * 8] = _wrap_idx(fa2)
        idxB2[c, :, t * CB * 8:(t + 1) * CB * 8] = _wrap_idx(fb2)
        # dstloc: chunks 0..CA-1 = A slots, CA..CPT-1 = B slots
        dl = np.full((CPT, P), 200.0, np.float64)
        ia = np.arange(len(esA))
        dl[ia // P, ia % P] = slA
        ib = np.arange(len(esB))
        dl[CA + ib // P, ib % P] = slB
        dstloc[c, :, t * CPT:(t + 1) * CPT] = dl.T.astype(BF16)
        ad = np.full(P, n_nodes, np.int64)    # pad slots -> dummy row
        ad[:cnt] = n0 + np.arange(cnt)
        adoffs[c, :, t] = ad
        t2bias[c, :, 2 * t:2 * t + 2] = 0.0

    meta = dict(idxA1=idxA1, idxB1=idxB1, idxA2=idxA2, idxB2=idxB2,
                dstloc=dstloc, adoffs=adoffs, t2bias=t2bias)
    shapes = dict(T=T, NT=NT, NCH=NCH, nrow1=nrow1, nrow2=nrow2,
                  THR=THR, RB=RB)
    return meta, shapes, nstart, ncnt


def _host_tables(x, W1, att_src1, att_dst1, W2, att_src2, att_dst2, shapes):
    n_nodes = x.shape[0]
    nrow1 = shapes['nrow1']

    t1x = np.zeros((nrow1, 2 * ROW), BF16)
    t1x[:n_nodes, :F] = x.astype(BF16)

    xT = np.zeros((P, nrow1), BF16)
    xT[:, :n_nodes] = x.astype(BF16).T

    W1r = W1.reshape(F, H, C)
    wsd = np.zeros((P, 8), BF16)
    wsd[:, 0:4] = np.einsum('fhc,hc->fh', W1r, att_src1).astype(BF16)
    wsd[:, 4:8] = np.einsum('fhc,hc->fh', W1r, att_dst1).astype(BF16)

    w1 = W1.astype(BF16)                                   # [128, 256]
    ws2 = (W2 @ att_src2[0])[:, None]
    wd2 = (W2 @ att_dst2[0])[:, None]
    w2e = np.concatenate([W2, ws2, wd2], axis=1).astype(BF16)  # [256, 42]
    w2e_packed = np.concatenate([w2e[0:P], w2e[P:2 * P]], axis=1)  # [128, 84]

    iota_k = np.tile(np.arange(P, dtype=BF16), CPT)[None, :].repeat(P, 0)
    ident = np.eye(P, dtype=BF16)
    return dict(t1x=t1x, xT=xT, wsd=wsd, w1=w1, w2e=w2e_packed,
                iota_k=np.ascontiguousarray(iota_k), ident=ident)


# ------------------------------------------------------------- device program

def _build(shapes, n_nodes, debug=False, phases='ABCD'):
    B_STOP = globals()['B_STOP']
    NT, NCH = shapes['NT'], shapes['NCH']
    nrow1, nrow2 = shapes['nrow1'], shapes['nrow2']
    THR, RB = shapes['THR'], shapes['RB']
    NAT = nrow1
    bf = mybir.dt.bfloat16
    f32 = mybir.dt.float32
    i32 = mybir.dt.int32
    i16 = mybir.dt.int16
    AG_CHUNKS = 4
    assert NT % AG_CHUNKS == 0 or True

    nc = bacc.Bacc('TRN2', target_bir_lowering=False, debug=False,
                   num_devices=NC)

    def inp(name, shape, dt):
        return nc.dram_tensor(name, list(shape), dt, kind='ExternalInput').ap()

    t1x = inp('t1x', (nrow1, 2 * ROW), bf)
    xT = inp('xT', (P, nrow1), bf)
    wsd = inp('wsd', (P, 8), bf)
    w1 = inp('w1', (P, H * C), bf)
    w2e = inp('w2e', (P, 2 * 42), bf)
    iota_k = inp('iota_k', (P, CPT * P), bf)
    ident = inp('ident', (P, P), bf)
    b1bc = inp('b1bc', (P, H * C), f32)
    b2bc = inp('b2bc', (P, 40), f32)
    t2bias = inp('t2bias', (P, NT * 2), f32)
    m_idxA1 = inp('idxA1', (P, NT * CA * 8), i16)
    m_idxB1 = inp('idxB1', (P, NT * CB * 8), i16)
    m_idxA2 = inp('idxA2', (P, NT * CA * 8), i16)
    m_idxB2 = inp('idxB2', (P, NT * CB * 8), i16)
    m_dstloc = inp('dstloc', (P, NCH), bf)
    m_adoffs = inp('adoffs', (P, NT), i32)

    out_d = nc.dram_tensor('out', [NT * P, 40], f32, kind='ExternalOutput').ap()

    adtab = nc.dram_tensor('adtab', [NAT, 4], bf).ap()     # [a_d(4)]

    dbg = {}

    with tile.TileContext(nc) as tc, ExitStack() as ctx:
        nc.gpsimd.load_library(library_config.mlp)
        dram = ctx.enter_context(tc.tile_pool(name='dram', bufs=1, space='DRAM'))
        t2_local = dram.tile([NT * P, ROW], bf)
        t2_full = dram.tile([nrow2, ROW], bf, addr_space='Shared')

        consts = ctx.enter_context(tc.tile_pool(name='consts', bufs=1))
        meta = ctx.enter_context(tc.tile_pool(name='meta', bufs=1))

        s_wsd = consts.tile([P, 8], bf)
        nc.sync.dma_start(out=s_wsd, in_=wsd)
        s_w1 = consts.tile([P, H * C], bf)
        nc.sync.dma_start(out=s_w1, in_=w1)
        s_w2e = consts.tile([P, 2 * 42], bf)
        nc.sync.dma_start(out=s_w2e, in_=w2e)
        s_iota = consts.tile([P, CPT * P], bf)
        nc.sync.dma_start(out=s_iota, in_=iota_k)
        s_ident = consts.tile([P, P], bf)
        nc.sync.dma_start(out=s_ident, in_=ident)
        s_b1 = consts.tile([P, H * C], f32)
        nc.sync.dma_start(out=s_b1, in_=b1bc)
        s_b2 = consts.tile([P, 40], f32)
        nc.sync.dma_start(out=s_b2, in_=b2bc)
        s_t2bias = consts.tile([P, NT * 2], f32)
        nc.sync.dma_start(out=s_t2bias, in_=t2bias)

        s_idxA1 = meta.tile([P, NT * CA * 8], i16)
        nc.sync.dma_start(out=s_idxA1, in_=m_idxA1)
        s_idxB1 = meta.tile([P, NT * CB * 8], i16)
        nc.sync.dma_start(out=s_idxB1, in_=m_idxB1)
        s_idxA2 = meta.tile([P, NT * CA * 8], i16)
        nc.sync.dma_start(out=s_idxA2, in_=m_idxA2)
        s_idxB2 = meta.tile([P, NT * CB * 8], i16)
        nc.sync.dma_start(out=s_idxB2, in_=m_idxB2)
        s_dstloc = meta.tile([P, NCH], bf)
        nc.sync.dma_start(out=s_dstloc, in_=m_dstloc)
        s_adoffs = meta.tile([P, NT], i32)
        nc.sync.dma_start(out=s_adoffs, in_=m_adoffs)

        # ---------------- phase A: a_s (astab) and a_d (adtab) tables ------
        GA = 8
        n_a_tiles = nrow1 // P
        with tc.tile_pool(name='pa', bufs=2) as pa, \
             tc.tile_pool(name='pa_ps', bufs=2, space='PSUM') as pa_ps:
            for t0 in range(0, n_a_tiles if 'A' in phases else 0, GA):
                g = min(GA, n_a_tiles - t0)
                xt = pa.tile([P, GA * P], bf, tag='xt')
                nc.sync.dma_start(out=xt[:, :g * P],
                                  in_=xT[:, t0 * P:(t0 + g) * P])
                ps = pa_ps.tile([P, GA * 8], f32, tag='ps')
                for j in range(g):
                    nc.tensor.matmul(out=ps[:, j * 8:(j + 1) * 8],
                                     lhsT=xt[:, j * P:(j + 1) * P],
                                     rhs=s_wsd, start=True, stop=True)
                sa = pa.tile([P, GA * 4], bf, tag='sa')
                nc.vector.tensor_copy(
                    out=sa[:, :g * 4].rearrange('p (j e) -> p j e', e=4),
                    in_=ps[:, :g * 8].rearrange('p (j e) -> p j e', e=8)[:, :, 0:4])
                sd = pa.tile([P, GA * 4], bf, tag='sd')
                nc.vector.tensor_copy(
                    out=sd[:, :g * 4].rearrange('p (j e) -> p j e', e=4),
                    in_=ps[:, :g * 8].rearrange('p (j e) -> p j e', e=8)[:, :, 4:8])
                as_ap = bass.AP(tensor=t1x.tensor,
                                offset=t0 * P * 2 * ROW + F,
                                ap=[[2 * ROW, P], [P * 2 * ROW, g], [1, 4]])
                nc.sync.dma_start(
                    out=as_ap,
                    in_=sa[:, :g * 4].rearrange('p (j e) -> p j e', e=4))
                ad_ap = bass.AP(tensor=adtab.tensor,
                                offset=t0 * P * 4,
                                ap=[[4, P], [P * 4, g], [1, 4]])
                nc.sync.dma_start(
                    out=ad_ap,
                    in_=sd[:, :g * 4].rearrange('p (j e) -> p j e', e=4))

        if 'Z' in phases:   # minimal: write zeros to out
            with tc.tile_pool(name='pz', bufs=1) as pz:
                zt = pz.tile([P, 40], f32)
                nc.vector.memset(zt, 0.0)
                for t in range(NT):
                    nc.sync.dma_start(out=out_d[t * P:(t + 1) * P, :], in_=zt)

        # ---------------- phase B: layer-1 edges + tile epilogue -----------
        with tc.tile_pool(name='pb', bufs=2) as pb, \
             tc.tile_pool(name='pb_rhs', bufs=2) as pb_rhs, \
             tc.tile_pool(name='pb_ep', bufs=2) as pb_ep, \
             tc.tile_pool(name='ps_acc', bufs=1, space='PSUM') as ps_acc, \
             tc.tile_pool(name='ps_sm', bufs=2, space='PSUM') as ps_sm, \
             tc.tile_pool(name='ps_ep', bufs=1, space='PSUM') as ps_ep:
            for t in range(NT if 'B' in phases else 0):
                c0 = t * CPT
                gx = pb.tile([P, CPT, 2 * ROW], bf, tag='gx')
                nc.gpsimd.dma_gather(gx[:, 0:CA, :], t1x,
                                     s_idxA1[:, t * CA * 8:(t + 1) * CA * 8],
                                     CA * P, CA * P, 2 * ROW,
                                     single_packet=False)
                nc.gpsimd.dma_gather(gx[:, CA:CPT, :], t1x[THR:nrow1, :],
                                     s_idxB1[:, t * CB * 8:(t + 1) * CB * 8],
                                     CB * P, CB * P, 2 * ROW,
                                     single_packet=False)
                adt = pb.tile([P, 4], bf, tag='adt')
                nc.gpsimd.indirect_dma_start(
                    out=adt, out_offset=None, in_=adtab,
                    in_offset=bass.IndirectOffsetOnAxis(
                        ap=s_adoffs[:, t:t + 1], axis=0))
                if B_STOP < 2:
                    continue
                # one-hot S01 for the whole tile
                s01 = pb.tile([P, CPT * P], bf, tag='s01')
                nc.vector.tensor_tensor(
                    out=s01.rearrange('p (k j) -> p k j', j=P),
                    in0=s_iota.rearrange('p (k j) -> p k j', j=P),
                    in1=s_dstloc[:, c0:c0 + CPT].to_broadcast([P, CPT, P]),
                    op=mybir.AluOpType.is_equal)
                # per-chunk: transpose S01 -> expand a_d -> scores
                sstage = pb.tile([P, CPT * 4], bf, tag='sstage')
                pss = [ps_acc.tile([P, P], f32, tag=f'ph{h}',
                                   name=f'ph{h}')
                       for h in range(H)]
                den_ps = ps_acc.tile([P, 8], f32, tag='den')
                w = pb.tile([P, CPT * 4], bf, tag='w')
                if B_STOP < 3:
                    continue
                for j in range(CPT):
                    psT = ps_sm.tile([P, P], bf, tag='psT', name='psT')
                    nc.tensor.transpose(out=psT, in_=s01[:, j * P:(j + 1) * P],
                                        identity=s_ident)
                    s01T = pb_rhs.tile([P, P], bf, tag='s01T')
                    nc.vector.tensor_copy(out=s01T, in_=psT)
                    adp = ps_sm.tile([P, 4], f32, tag='psT', name='adp')
                    nc.tensor.matmul(out=adp, lhsT=s01T, rhs=adt,
                                     start=True, stop=True)
                    nc.vector.tensor_tensor(
                        out=sstage[:, j * 4:(j + 1) * 4],
                        in0=gx[:, j, F:F + 4], in1=adp,
                        op=mybir.AluOpType.add)
                if B_STOP < 4:
                    continue
                st = pb.tile([P, CPT * 4], bf, tag='st')
                nc.vector.tensor_scalar(out=st, in0=sstage, scalar1=NEG_SLOPE,
                                        scalar2=None, op0=mybir.AluOpType.mult)
                sl = pb.tile([P, CPT * 4], bf, tag='sl')
                nc.vector.tensor_tensor(out=sl, in0=st, in1=sstage,
                                        op=mybir.AluOpType.max)
                nc.scalar.activation(w, sl, mybir.ActivationFunctionType.Exp)
                # rhs_h = w_h * x ; accumulate
                rhs = []
                w3 = w.rearrange('p (k e) -> p k e', e=4)
                for h in range(H):
                    r = pb_rhs.tile([P, CPT * P], bf, tag=f'rhs{h}')
                    nc.vector.tensor_tensor(
                        out=r.rearrange('p (k j) -> p k j', j=P),
                        in0=gx[:, :, 0:F],
                        in1=w3[:, :, h:h + 1].to_broadcast([P, CPT, P]),
                        op=mybir.AluOpType.mult)
                    rhs.append(r)
                if B_STOP < 5:
                    continue
                for j in range(CPT):
                    for h in range(H):
                        nc.tensor.matmul(
                            out=pss[h][:, 0:128],
                            lhsT=s01[:, j * P:(j + 1) * P],
                            rhs=rhs[h][:, j * P:(j + 1) * P],
                            start=(j == 0), stop=(j == CPT - 1))
                    nc.tensor.matmul(
                        out=den_ps[:, 0:4],
                        lhsT=s01[:, j * P:(j + 1) * P],
                        rhs=w[:, j * 4:(j + 1) * 4],
                        start=(j == 0), stop=(j == CPT - 1))
                # epilogue
                if B_STOP < 6:
                    continue
                den = pb_ep.tile([P, 4], f32, tag='den4')
                nc.vector.tensor_scalar(out=den, in0=den_ps[:, 0:4],
                                        scalar1=1e-20, scalar2=None,
                                        op0=mybir.AluOpType.max)
                rec = pb_ep.tile([P, 4], f32, tag='rec')
                nc.vector.reciprocal(out=rec, in_=den)
                out1 = ps_ep.tile([P, H * C], f32, tag='ep')
                for h in range(H):
                    an = pb_ep.tile([P, P], bf, tag=f'an{h}')
                    nc.scalar.activation(an, pss[h][:, 0:128],
                                         mybir.ActivationFunctionType.Copy,
                                         scale=rec[:, h:h + 1])
                    psT2 = ps_sm.tile([P, P], bf, tag='psT', name='psT2')
                    nc.tensor.transpose(out=psT2, in_=an, identity=s_ident)
                    anT = pb_ep.tile([P, P], bf, tag=f'anT{h}')
                    nc.vector.tensor_copy(out=anT, in_=psT2)
                    nc.tensor.matmul(out=out1[:, h * C:(h + 1) * C],
                                     lhsT=anT,
                                     rhs=s_w1[:, h * C:(h + 1) * C],
                                     start=True, stop=True)
                zb = pb_ep.tile([P, H * C], f32, tag='zb')
                nc.vector.tensor_tensor(out=zb, in0=out1, in1=s_b1,
                                        op=mybir.AluOpType.add)
                zr = pb_ep.tile([P, H * C], f32, tag='zr')
                nc.scalar.activation(zr, zb, mybir.ActivationFunctionType.Relu)
                zm = pb_ep.tile([P, H * C], f32, tag='zm')
                nc.vector.tensor_tensor(out=zm, in0=zb, in1=zr,
                                        op=mybir.AluOpType.subtract)
                ze = pb_ep.tile([P, H * C], f32, tag='ze')
                nc.scalar.activation(ze, zm, mybir.ActivationFunctionType.Exp)
                hb = pb_ep.tile([P, H * C], bf, tag='hb')
                nc.vector.scalar_tensor_tensor(out=hb, in0=zr, scalar=-1.0,
                                               in1=ze,
                                               op0=mybir.AluOpType.add,
                                               op1=mybir.AluOpType.add)
                xw2 = ps_ep.tile([P, 42], f32, tag='ep', name='xw2')
                for kk in range(2):
                    psT3 = ps_sm.tile([P, P], bf, tag='psT', name='psT3')
                    nc.tensor.transpose(out=psT3, in_=hb[:, kk * P:(kk + 1) * P],
                                        identity=s_ident)
                    hT = pb_ep.tile([P, P], bf, tag=f'hT{kk}')
                    nc.vector.tensor_copy(out=hT, in_=psT3)
                    nc.tensor.matmul(out=xw2, lhsT=hT,
                                     rhs=s_w2e[:, kk * 42:(kk + 1) * 42],
                                     start=(kk == 0), stop=(kk == 1))
                t2r = pb_ep.tile([P, ROW], bf, tag='t2r')
                nc.vector.memset(t2r[:, 42:ROW], 0.0)
                nc.vector.tensor_copy(out=t2r[:, 0:40], in_=xw2[:, 0:40])
                nc.vector.tensor_tensor(out=t2r[:, 40:42], in0=xw2[:, 40:42],
                                        in1=s_t2bias[:, 2 * t:2 * t + 2],
                                        op=mybir.AluOpType.add)
                nc.sync.dma_start(out=t2_local[t * P:(t + 1) * P, :], in_=t2r)

        # ---------------- phase C: allgather (chunked) ----------------
        CHK = NT // AG_CHUNKS if NT % AG_CHUNKS == 0 else NT
        nch = NT // CHK
        for a in range(nch if 'C' in phases else 0):
            nc.gpsimd.collective_compute(
                'AllGather', mybir.AluOpType.bypass,
                ins=[t2_local[a * CHK * P:(a + 1) * CHK * P, :]],
                outs=[t2_full.rearrange('(c r) e -> c r e', c=NC)
                      [:, a * CHK * P:(a + 1) * CHK * P, :]],
                replica_groups=[list(range(NC))])

        # ---------------- phase D: layer-2 edges + log_softmax ------------
        with tc.tile_pool(name='pd', bufs=2) as pd, \
             tc.tile_pool(name='pd_rhs', bufs=2) as pd_rhs, \
             tc.tile_pool(name='pd_ep', bufs=2) as pd_ep, \
             tc.tile_pool(name='ps2', bufs=2, space='PSUM') as ps2, \
             tc.tile_pool(name='ps2_sm', bufs=2, space='PSUM') as ps2_sm:
            for t in range(NT if 'D' in phases else 0):
                c0 = t * CPT
                g2 = pd.tile([P, CPT, ROW], bf, tag='g2')
                nc.gpsimd.dma_gather(g2[:, 0:CA, :], t2_full[:, :],
                                     s_idxA2[:, t * CA * 8:(t + 1) * CA * 8],
                                     CA * P, CA * P, ROW, single_packet=False)
                nc.gpsimd.dma_gather(g2[:, CA:CPT, :], t2_full[RB:nrow2, :],
                                     s_idxB2[:, t * CB * 8:(t + 1) * CB * 8],
                                     CB * P, CB * P, ROW, single_packet=False)
                ad2 = pd.tile([P, 2], bf, tag='ad2')
                nc.sync.dma_start(out=ad2,
                                  in_=t2_local[t * P:(t + 1) * P, 40:42])
                s01 = pd.tile([P, CPT * P], bf, tag='s01b')
                nc.vector.tensor_tensor(
                    out=s01.rearrange('p (k j) -> p k j', j=P),
                    in0=s_iota.rearrange('p (k j) -> p k j', j=P),
                    in1=s_dstloc[:, c0:c0 + CPT].to_broadcast([P, CPT, P]),
                    op=mybir.AluOpType.is_equal)
                sstage = pd.tile([P, CPT], bf, tag='sst2')
                acc = ps2.tile([P, 40], f32, tag='acc2')
                den_ps = ps2.tile([P, 8], f32, tag='den2p')
                for j in range(CPT):
                    psT = ps2_sm.tile([P, P], bf, tag='psT2', name='psTD')
                    nc.tensor.transpose(out=psT, in_=s01[:, j * P:(j + 1) * P],
                                        identity=s_ident)
                    s01T = pd_rhs.tile([P, P], bf, tag='s01T2')
                    nc.vector.tensor_copy(out=s01T, in_=psT)
                    adp = ps2_sm.tile([P, 1], f32, tag='psT2', name='adpD')
                    nc.tensor.matmul(out=adp, lhsT=s01T, rhs=ad2[:, 1:2],
                                     start=True, stop=True)
                    nc.vector.tensor_tensor(
                        out=sstage[:, j:j + 1],
                        in0=g2[:, j, 40:41], in1=adp,
                        op=mybir.AluOpType.add)
                st = pd.tile([P, CPT], bf, tag='st2')
                nc.vector.tensor_scalar(out=st, in0=sstage, scalar1=NEG_SLOPE,
                                        scalar2=None, op0=mybir.AluOpType.mult)
                sl = pd.tile([P, CPT], bf, tag='sl2')
                nc.vector.tensor_tensor(out=sl, in0=st, in1=sstage,
                                        op=mybir.AluOpType.max)
                w = pd.tile([P, CPT], bf, tag='w2')
                nc.scalar.activation(w, sl, mybir.ActivationFunctionType.Exp)
                r = pd_rhs.tile([P, CPT * 40], bf, tag='rhs2')
                nc.vector.tensor_tensor(
                    out=r.rearrange('p (k j) -> p k j', j=40),
                    in0=g2[:, :, 0:40],
                    in1=w.rearrange('p (k o) -> p k o', o=1)
                    .to_broadcast([P, CPT, 40]),
                    op=mybir.AluOpType.mult)
                for j in range(CPT):
                    nc.tensor.matmul(out=acc[:, :],
                                     lhsT=s01[:, j * P:(j + 1) * P],
                                     rhs=r[:, j * 40:(j + 1) * 40],
                                     start=(j == 0), stop=(j == CPT - 1))
                    nc.tensor.matmul(out=den_ps[:, 0:1],
                                     lhsT=s01[:, j * P:(j + 1) * P],
                                     rhs=w[:, j:j + 1],
                                     start=(j == 0), stop=(j == CPT - 1))
                den = pd_ep.tile([P, 1], f32, tag='den2')
                nc.vector.tensor_scalar(out=den, in0=den_ps[:, 0:1],
                                        scalar1=1e-20, scalar2=None,
                                        op0=mybir.AluOpType.max)
                rec = pd_ep.tile([P, 1], f32, tag='rec2')
                nc.vector.reciprocal(out=rec, in_=den)
                o = pd_ep.tile([P, 40], f32, tag='o')
                nc.scalar.activation(o, acc[:, 0:40],
                                     mybir.ActivationFunctionType.Copy,
                                     scale=rec)
                ob = pd_ep.tile([P, 40], f32, tag='ob')
                nc.vector.tensor_tensor(out=ob, in0=o, in1=s_b2,
                                        op=mybir.AluOpType.add)
                ex = pd_ep.tile([P, 40], f32, tag='ex')
                sm = pd_ep.tile([P, 1], f32, tag='sm')
                nc.scalar.activation(ex, ob, mybir.ActivationFunctionType.Exp,
                                     accum_out=sm)
                rsm = pd_ep.tile([P, 1], f32, tag='rsm')
                nc.vector.reciprocal(out=rsm, in_=sm)
                nlg = pd_ep.tile([P, 1], f32, tag='nlg')
                nc.scalar.activation(nlg, rsm,
                                     mybir.ActivationFunctionType.Ln)
                fin = pd_ep.tile([P, 40], f32, tag='fin')
                nc.scalar.activation(fin, ob,
                                     mybir.ActivationFunctionType.Identity,
                                     bias=nlg)
                nc.sync.dma_start(out=out_d[t * P:(t + 1) * P, :], in_=fin)

    nc.compile()
    return nc


# ----------------------------------------------------------------- entry

_CACHE = {}


def prepare(x, edge_index, W1, att_src1, att_dst1, b1, W2, att_src2, att_dst2,
            b2, build=True, debug=False):
    x = np.asarray(x, F32)
    edge_index = np.asarray(edge_index)
    n_nodes = x.shape[0]

    meta, shapes, nstart, ncnt = _prep(edge_index, n_nodes)
    tables = _host_tables(x, np.asarray(W1, F32), np.asarray(att_src1, F32),
                          np.asarray(att_dst1, F32), np.asarray(W2, F32),
                          np.asarray(att_src2, F32), np.asarray(att_dst2, F32),
                          shapes)
    nc = None
    if build:
        key = (shapes['NT'], shapes['THR'], shapes['RB'], n_nodes, debug)
        if key not in _CACHE:
            _CACHE[key] = _build(shapes, n_nodes, debug=debug)
        nc = _CACHE[key]

    b1bc = np.broadcast_to(np.asarray(b1, F32), (P, H * C)).copy()
    b2bc = np.broadcast_to(np.asarray(b2, F32), (P, 40)).copy()

    in_maps = []
    for c in range(NC):
        in_maps.append(dict(
            t1x=tables['t1x'], xT=tables['xT'], wsd=tables['wsd'],
            w1=tables['w1'], w2e=tables['w2e'], iota_k=tables['iota_k'],
            ident=tables['ident'], b1bc=b1bc, b2bc=b2bc,
            t2bias=meta['t2bias'][c],
            idxA1=meta['idxA1'][c], idxB1=meta['idxB1'][c],
            idxA2=meta['idxA2'][c], idxB2=meta['idxB2'][c],
            dstloc=meta['dstloc'][c], adoffs=meta['adoffs'][c],
        ))
    return dict(nc=nc, in_maps=in_maps, shapes=shapes, nstart=nstart,
                ncnt=ncnt, n_nodes=n_nodes)


def assemble(ctx_run, outs):
    NT = ctx_run['shapes']['NT']
    nstart, ncnt = ctx_run['nstart'], ctx_run['ncnt']
    out = np.zeros((ctx_run['n_nodes'], 40), F32)
    for c in range(NC):
        oc = outs[c]['out']
        for t in range(NT):
            cnt = int(ncnt[c, t])
            if cnt == 0:
                continue
            n0 = int(nstart[c, t])
            out[n0:n0 + cnt] = oc[t * P:t * P + cnt]
    return out


def kernel(x, edge_index, W1, att_src1, att_dst1, b1, W2, att_src2, att_dst2, b2):
    ctx_run = prepare(x, edge_index, W1, att_src1, att_dst1, b1,
                      W2, att_src2, att_dst2, b2)
    res = run_bass_kernel_spmd(ctx_run['nc'], ctx_run['in_maps'],
                               list(range(NC)))
    return assemble(ctx_run, res.results)


# revision 33
# speedup vs baseline: 1.6552x; 1.1329x over previous
"""Trainium2 Bass kernel for 2-layer GAT (nn_GAT_90460601188538).

Strategy: edges sorted by destination; destination nodes greedily packed
into 128-slot tiles; tiles split contiguously across 8 cores. Per
edge-chunk of 128, a one-hot selection matrix (iota == dst_slot) turns the
segmented softmax-sum and scatter-add into PE matmuls accumulating in
PSUM. Softmax runs without max subtraction (scores are O(1)) as
unnormalized sums plus one divide per node. Edge-source features are
fetched with dma_gather (int16 indices), so every table is split at one
global node threshold THR into two halves gathered separately; each tile's
chunks are partitioned into group A (src < THR) and group B. Pad edge
slots point at row 0 with dst slot 200, whose one-hot row is all zero, so
they contribute nothing. Layer-2 features are exchanged with an AllGather.
"""
import sys
sys.path.insert(0, '/opt/trn_rl_repo')
from contextlib import ExitStack

import numpy as np
import ml_dtypes

import concourse.bacc as bacc
import concourse.tile as tile
from concourse import bass, mybir, library_config
from concourse.bass_utils import run_bass_kernel_spmd

BF16 = ml_dtypes.bfloat16
F32 = np.float32

P = 128
NC = 8
CA = 12               # group-A chunks per tile (src < THR)
CB = 8                # group-B chunks per tile (src >= THR)
CPT = CA + CB
NEG_SLOPE = 0.2
NEG_BIG = -10000.0
ROW = 128             # table row elems (bf16) = 256B
H, C, F = 4, 64, 128
THR_CAP = 30720       # node split threshold cap (int16 table indexing)
B_STOP = 6            # debug: truncate phase-B body (1..6)


# ----------------------------------------------------------------- host prep

def _wrap_idx(flat):
    """[n] int -> dma_gather layout [128, n//16] int16 (16-wrap, replicated)."""
    n = flat.shape[0]
    out = np.zeros((P, n // 16), np.int16)
    cols = flat.reshape(n // 16, 16).T.astype(np.int16)   # [16, n//16]
    for rep in range(8):
        out[rep * 16:(rep + 1) * 16, :] = cols
    return out


def _prep(edge_index, n_nodes):
    src = np.concatenate([edge_index[0], np.arange(n_nodes, dtype=np.int64)])
    dst = np.concatenate([edge_index[1], np.arange(n_nodes, dtype=np.int64)])
    perm = np.argsort(dst, kind='stable')
    src_s = src[perm].astype(np.int64)
    dst_s = dst[perm].astype(np.int64)
    deg = np.bincount(dst_s, minlength=n_nodes)

    THR = min(THR_CAP, n_nodes)   # node split threshold (A: src < THR)
    capA, capB = CA * P, CB * P

    # greedy pack consecutive nodes: <=P nodes, <=capA A-edges, <=capB B-edges
    isB = (src_s >= THR)
    degA = np.bincount(dst_s[~isB], minlength=n_nodes)
    degB = deg - degA
    cumA = np.concatenate([[0], np.cumsum(degA)])
    cumB = np.concatenate([[0], np.cumsum(degB)])
    tiles = []
    n = 0
    while n < n_nodes:
        hiA = int(np.searchsorted(cumA, cumA[n] + capA, side='right')) - 1 - n
        hiB = int(np.searchsorted(cumB, cumB[n] + capB, side='right')) - 1 - n
        cnt = max(1, min(hiA, hiB, P, n_nodes - n))
        assert cumA[n + cnt] - cumA[n] <= capA
        assert cumB[n + cnt] - cumB[n] <= capB
        tiles.append((n, cnt))
        n += cnt
    T = len(tiles)
    NT = (T + 1 + NC - 1) // NC      # >=1 pad tile overall
    NCH = NT * CPT

    nrow1 = ((n_nodes + P - 1) // P) * P
    nrow2 = NC * NT * P

    # node -> (global slot row, tile)
    node_row = np.zeros(n_nodes, np.int64)
    nstart = np.zeros((NC, NT), np.int64)
    ncnt = np.zeros((NC, NT), np.int64)
    for gi, (n0, cnt) in enumerate(tiles):
        c, t = gi // NT, gi % NT
        nstart[c, t] = n0
        ncnt[c, t] = cnt
        node_row[n0:n0 + cnt] = gi * P + np.arange(cnt)
    RB = int(node_row[THR]) if THR < n_nodes else max(0, nrow2 - P)
    assert RB < 32768 and nrow2 - RB <= 32768, (RB, nrow2)
    assert nrow1 - THR <= 32768, (THR, nrow1)

    # per-core streams
    idxA1 = np.zeros((NC, P, NT * CA * 8), np.int16)
    idxB1 = np.zeros((NC, P, NT * CB * 8), np.int16)
    idxA2 = np.zeros((NC, P, NT * CA * 8), np.int16)
    idxB2 = np.zeros((NC, P, NT * CB * 8), np.int16)
    dstloc = np.full((NC, P, NCH), BF16(200.0), BF16)
    adoffs = np.zeros((NC, P, NT), np.int32)      # slot node ids (a_d tile)
    t2bias = np.full((NC, P, NT * 2), NEG_BIG, F32)

    # edge ranges per tile
    epos = 0
    edge_of_tile = []
    for (n0, cnt) in tiles:
        e0 = epos
        ecnt = int(deg[n0:n0 + cnt].sum())
        edge_of_tile.append((e0, ecnt))
        epos += ecnt

    for gi, (n0, cnt) in enumerate(tiles):
        c, t = gi // NT, gi % NT
        e0, ecnt = edge_of_tile[gi]
        es = src_s[e0:e0 + ecnt]
        ed = dst_s[e0:e0 + ecnt]
        sl = (ed - n0).astype(np.int64)
        selB = es >= THR
        esA, slA = es[~selB], sl[~selB]
        esB, slB = es[selB], sl[selB]
        fa = np.zeros(capA, np.int64)
        fa[:len(esA)] = esA
        fb = np.zeros(capB, np.int64)
        fb[:len(esB)] = esB - THR
        idxA1[c, :, t * CA * 8:(t + 1) * CA * 8] = _wrap_idx(fa)
        idxB1[c, :, t * CB * 8:(t + 1) * CB * 8] = _wrap_idx(fb)
        fa2 = np.zeros(capA, np.int64)
        fa2[:len(esA)] = node_row[esA]
        fb2 = np.zeros(capB, np.int64)
        fb2[:len(esB)] = node_row[esB] - RB
        idxA2[c, :, t * CA * 8:(t + 1) * CA * 8] = _wrap_idx(fa2)
        idxB2[c, :, t * CB * 8:(t + 1) * CB * 8] = _wrap_idx(fb2)
        # dstloc: chunks 0..CA-1 = A slots, CA..CPT-1 = B slots
        dl = np.full((CPT, P), 200.0, np.float64)
        ia = np.arange(len(esA))
        dl[ia // P, ia % P] = slA
        ib = np.arange(len(esB))
        dl[CA + ib // P, ib % P] = slB
        dstloc[c, :, t * CPT:(t + 1) * CPT] = dl.T.astype(BF16)
        ad = np.full(P, n_nodes, np.int64)    # pad slots -> dummy row
        ad[:cnt] = n0 + np.arange(cnt)
        adoffs[c, :, t] = ad
        t2bias[c, :, 2 * t:2 * t + 2] = 0.0

    # host-built transposed one-hot: s01T[d, (t,j,e)] = 1 iff dst slot of
    # edge slot (t,j,e) == d; lhsT for broadcasting a_d to edge slots.
    dvals = np.arange(P, dtype=np.float64)
    s01T = np.zeros((NC, P, NT * CPT * P), BF16)
    for c in range(NC):
        # dstloc[c] is [P(e), NCH(t,j)] -> oh [d, t*CPT+j, e]
        oh = (dstloc[c].astype(np.float64).T[None, :, :] == dvals[:, None, None])
        s01T[c] = np.ascontiguousarray(oh.transpose(0, 1, 2)).reshape(P, -1).astype(BF16)

    meta = dict(idxA1=idxA1, idxB1=idxB1, idxA2=idxA2, idxB2=idxB2,
                dstloc=dstloc, adoffs=adoffs, t2bias=t2bias, s01T=s01T)
    shapes = dict(T=T, NT=NT, NCH=NCH, nrow1=nrow1, nrow2=nrow2,
                  THR=THR, RB=RB)
    return meta, shapes, nstart, ncnt


def _host_tables(x, W1, att_src1, att_dst1, W2, att_src2, att_dst2, shapes):
    n_nodes = x.shape[0]
    nrow1 = shapes['nrow1']

    t1x = np.zeros((nrow1, 2 * ROW), BF16)
    t1x[:n_nodes, :F] = x.astype(BF16)

    xT = np.zeros((P, nrow1), BF16)
    xT[:, :n_nodes] = x.astype(BF16).T

    W1r = W1.reshape(F, H, C)
    wsd = np.zeros((P, 8), BF16)
    wsd[:, 0:4] = np.einsum('fhc,hc->fh', W1r, att_src1).astype(BF16)
    wsd[:, 4:8] = np.einsum('fhc,hc->fh', W1r, att_dst1).astype(BF16)

    w1 = W1.astype(BF16)                                   # [128, 256]
    ws2 = (W2 @ att_src2[0])[:, None]
    wd2 = (W2 @ att_dst2[0])[:, None]
    w2e = np.concatenate([W2, ws2, wd2], axis=1).astype(BF16)  # [256, 42]
    w2e_packed = np.concatenate([w2e[0:P], w2e[P:2 * P]], axis=1)  # [128, 84]

    iota_k = np.tile(np.arange(P, dtype=BF16), CPT)[None, :].repeat(P, 0)
    ident = np.eye(P, dtype=BF16)
    return dict(t1x=t1x, xT=xT, wsd=wsd, w1=w1, w2e=w2e_packed,
                iota_k=np.ascontiguousarray(iota_k), ident=ident)


# ------------------------------------------------------------- device program

def _build(shapes, n_nodes, debug=False, phases='ABCD'):
    B_STOP = globals()['B_STOP']
    NT, NCH = shapes['NT'], shapes['NCH']
    nrow1, nrow2 = shapes['nrow1'], shapes['nrow2']
    THR, RB = shapes['THR'], shapes['RB']
    NAT = nrow1
    bf = mybir.dt.bfloat16
    f32 = mybir.dt.float32
    i32 = mybir.dt.int32
    i16 = mybir.dt.int16
    AG_CHUNKS = 4
    assert NT % AG_CHUNKS == 0 or True

    nc = bacc.Bacc('TRN2', target_bir_lowering=False, debug=False,
                   num_devices=NC)

    def inp(name, shape, dt):
        return nc.dram_tensor(name, list(shape), dt, kind='ExternalInput').ap()

    t1x = inp('t1x', (nrow1, 2 * ROW), bf)
    xT = inp('xT', (P, nrow1), bf)
    wsd = inp('wsd', (P, 8), bf)
    w1 = inp('w1', (P, H * C), bf)
    w2e = inp('w2e', (P, 2 * 42), bf)
    iota_k = inp('iota_k', (P, CPT * P), bf)
    ident = inp('ident', (P, P), bf)
    b1bc = inp('b1bc', (P, H * C), f32)
    b2bc = inp('b2bc', (P, 40), f32)
    t2bias = inp('t2bias', (P, NT * 2), f32)
    m_idxA1 = inp('idxA1', (P, NT * CA * 8), i16)
    m_idxB1 = inp('idxB1', (P, NT * CB * 8), i16)
    m_idxA2 = inp('idxA2', (P, NT * CA * 8), i16)
    m_idxB2 = inp('idxB2', (P, NT * CB * 8), i16)
    m_dstloc = inp('dstloc', (P, NCH), bf)
    m_adoffs = inp('adoffs', (P, NT), i32)
    m_s01T = inp('s01T', (P, NT * CPT * P), bf)

    out_d = nc.dram_tensor('out', [NT * P, 40], f32, kind='ExternalOutput').ap()

    adtab = nc.dram_tensor('adtab', [NAT, 4], bf).ap()     # [a_d(4)]

    dbg = {}

    with tile.TileContext(nc) as tc, ExitStack() as ctx:
        nc.gpsimd.load_library(library_config.mlp)
        dram = ctx.enter_context(tc.tile_pool(name='dram', bufs=1, space='DRAM'))
        t2_local = dram.tile([NT * P, ROW], bf)
        t2_full = dram.tile([nrow2, ROW], bf, addr_space='Shared')

        consts = ctx.enter_context(tc.tile_pool(name='consts', bufs=1))
        meta = ctx.enter_context(tc.tile_pool(name='meta', bufs=1))

        s_wsd = consts.tile([P, 8], bf)
        nc.sync.dma_start(out=s_wsd, in_=wsd)
        s_w1 = consts.tile([P, H * C], bf)
        nc.sync.dma_start(out=s_w1, in_=w1)
        s_w2e = consts.tile([P, 2 * 42], bf)
        nc.sync.dma_start(out=s_w2e, in_=w2e)
        s_iota = consts.tile([P, CPT * P], bf)
        nc.sync.dma_start(out=s_iota, in_=iota_k)
        s_ident = consts.tile([P, P], bf)
        nc.sync.dma_start(out=s_ident, in_=ident)
        s_b1 = consts.tile([P, H * C], f32)
        nc.sync.dma_start(out=s_b1, in_=b1bc)
        s_b2 = consts.tile([P, 40], f32)
        nc.sync.dma_start(out=s_b2, in_=b2bc)
        s_t2bias = consts.tile([P, NT * 2], f32)
        nc.sync.dma_start(out=s_t2bias, in_=t2bias)

        s_idxA1 = meta.tile([P, NT * CA * 8], i16)
        nc.sync.dma_start(out=s_idxA1, in_=m_idxA1)
        s_idxB1 = meta.tile([P, NT * CB * 8], i16)
        nc.sync.dma_start(out=s_idxB1, in_=m_idxB1)
        s_idxA2 = meta.tile([P, NT * CA * 8], i16)
        nc.sync.dma_start(out=s_idxA2, in_=m_idxA2)
        s_idxB2 = meta.tile([P, NT * CB * 8], i16)
        nc.sync.dma_start(out=s_idxB2, in_=m_idxB2)
        s_dstloc = meta.tile([P, NCH], bf)
        nc.sync.dma_start(out=s_dstloc, in_=m_dstloc)
        s_adoffs = meta.tile([P, NT], i32)
        nc.sync.dma_start(out=s_adoffs, in_=m_adoffs)

        # ---------------- phase A: a_s (astab) and a_d (adtab) tables ------
        GA = 8
        n_a_tiles = nrow1 // P
        with tc.tile_pool(name='pa', bufs=2) as pa, \
             tc.tile_pool(name='pa_ps', bufs=2, space='PSUM') as pa_ps:
            for t0 in range(0, n_a_tiles if 'A' in phases else 0, GA):
                g = min(GA, n_a_tiles - t0)
                xt = pa.tile([P, GA * P], bf, tag='xt')
                nc.sync.dma_start(out=xt[:, :g * P],
                                  in_=xT[:, t0 * P:(t0 + g) * P])
                ps = pa_ps.tile([P, GA * 8], f32, tag='ps')
                for j in range(g):
                    nc.tensor.matmul(out=ps[:, j * 8:(j + 1) * 8],
                                     lhsT=xt[:, j * P:(j + 1) * P],
                                     rhs=s_wsd, start=True, stop=True)
                sa = pa.tile([P, GA * 4], bf, tag='sa')
                nc.vector.tensor_copy(
                    out=sa[:, :g * 4].rearrange('p (j e) -> p j e', e=4),
                    in_=ps[:, :g * 8].rearrange('p (j e) -> p j e', e=8)[:, :, 0:4])
                sd = pa.tile([P, GA * 4], bf, tag='sd')
                nc.vector.tensor_copy(
                    out=sd[:, :g * 4].rearrange('p (j e) -> p j e', e=4),
                    in_=ps[:, :g * 8].rearrange('p (j e) -> p j e', e=8)[:, :, 4:8])
                as_ap = bass.AP(tensor=t1x.tensor,
                                offset=t0 * P * 2 * ROW + F,
                                ap=[[2 * ROW, P], [P * 2 * ROW, g], [1, 4]])
                nc.sync.dma_start(
                    out=as_ap,
                    in_=sa[:, :g * 4].rearrange('p (j e) -> p j e', e=4))
                ad_ap = bass.AP(tensor=adtab.tensor,
                                offset=t0 * P * 4,
                                ap=[[4, P], [P * 4, g], [1, 4]])
                nc.sync.dma_start(
                    out=ad_ap,
                    in_=sd[:, :g * 4].rearrange('p (j e) -> p j e', e=4))

        if 'Z' in phases:   # minimal: write zeros to out
            with tc.tile_pool(name='pz', bufs=1) as pz:
                zt = pz.tile([P, 40], f32)
                nc.vector.memset(zt, 0.0)
                for t in range(NT):
                    nc.sync.dma_start(out=out_d[t * P:(t + 1) * P, :], in_=zt)

        # ---------------- phase B: layer-1 edges + tile epilogue -----------
        with tc.tile_pool(name='pb', bufs=2) as pb, \
             tc.tile_pool(name='pb_rhs', bufs=2) as pb_rhs, \
             tc.tile_pool(name='pb_ep', bufs=2) as pb_ep, \
             tc.tile_pool(name='ps_acc', bufs=1, space='PSUM') as ps_acc, \
             tc.tile_pool(name='ps_sm', bufs=2, space='PSUM') as ps_sm, \
             tc.tile_pool(name='ps_ep', bufs=1, space='PSUM') as ps_ep:
            for t in range(NT if 'B' in phases else 0):
                c0 = t * CPT
                gx = pb.tile([P, CPT, 2 * ROW], bf, tag='gx')
                nc.gpsimd.dma_gather(gx[:, 0:CA, :], t1x,
                                     s_idxA1[:, t * CA * 8:(t + 1) * CA * 8],
                                     CA * P, CA * P, 2 * ROW,
                                     single_packet=False)
                nc.gpsimd.dma_gather(gx[:, CA:CPT, :], t1x[THR:nrow1, :],
                                     s_idxB1[:, t * CB * 8:(t + 1) * CB * 8],
                                     CB * P, CB * P, 2 * ROW,
                                     single_packet=False)
                adt = pb.tile([P, 4], bf, tag='adt')
                nc.gpsimd.indirect_dma_start(
                    out=adt, out_offset=None, in_=adtab,
                    in_offset=bass.IndirectOffsetOnAxis(
                        ap=s_adoffs[:, t:t + 1], axis=0))
                if B_STOP < 2:
                    continue
                # one-hot S01 for the whole tile
                s01 = pb.tile([P, CPT * P], bf, tag='s01')
                nc.vector.tensor_tensor(
                    out=s01.rearrange('p (k j) -> p k j', j=P),
                    in0=s_iota.rearrange('p (k j) -> p k j', j=P),
                    in1=s_dstloc[:, c0:c0 + CPT].to_broadcast([P, CPT, P]),
                    op=mybir.AluOpType.is_equal)
                # scores: a_d broadcast via host s01T matmuls into PSUM,
                # then one strided add of the gathered a_s strip
                s01Tt = pb_rhs.tile([P, CPT * P], bf, tag='s01Tt')
                nc.sync.dma_start(
                    out=s01Tt,
                    in_=m_s01T[:, t * CPT * P:(t + 1) * CPT * P])
                sstage = pb.tile([P, CPT * 4], bf, tag='sstage')
                pss = [ps_acc.tile([P, P], f32, tag=f'ph{h}',
                                   name=f'ph{h}')
                       for h in range(H)]
                den_ps = ps_acc.tile([P, 8], f32, tag='den')
                w = pb.tile([P, CPT * 4], bf, tag='w')
                if B_STOP < 3:
                    continue
                sc_ps = ps_sm.tile([P, CPT * 4], f32, tag='psT', name='scps')
                for j in range(CPT):
                    nc.tensor.matmul(out=sc_ps[:, j * 4:(j + 1) * 4],
                                     lhsT=s01Tt[:, j * P:(j + 1) * P],
                                     rhs=adt, start=True, stop=True)
                nc.vector.tensor_tensor(
                    out=sstage.rearrange('p (k e) -> p k e', e=4),
                    in0=gx[:, :, F:F + 4],
                    in1=sc_ps.rearrange('p (k e) -> p k e', e=4),
                    op=mybir.AluOpType.add)
                if B_STOP < 4:
                    continue
                st = pb.tile([P, CPT * 4], bf, tag='st')
                nc.vector.tensor_scalar(out=st, in0=sstage, scalar1=NEG_SLOPE,
                                        scalar2=None, op0=mybir.AluOpType.mult)
                sl = pb.tile([P, CPT * 4], bf, tag='sl')
                nc.vector.tensor_tensor(out=sl, in0=st, in1=sstage,
                                        op=mybir.AluOpType.max)
                nc.scalar.activation(w, sl, mybir.ActivationFunctionType.Exp)
                # rhs_h = w_h * x ; accumulate
                rhs = []
                w3 = w.rearrange('p (k e) -> p k e', e=4)
                for h in range(H):
                    r = pb_rhs.tile([P, CPT * P], bf, tag=f'rhs{h}')
                    nc.vector.tensor_tensor(
                        out=r.rearrange('p (k j) -> p k j', j=P),
                        in0=gx[:, :, 0:F],
                        in1=w3[:, :, h:h + 1].to_broadcast([P, CPT, P]),
                        op=mybir.AluOpType.mult)
                    rhs.append(r)
                if B_STOP < 5:
                    continue
                for j in range(CPT):
                    for h in range(H):
                        nc.tensor.matmul(
                            out=pss[h][:, 0:128],
                            lhsT=s01[:, j * P:(j + 1) * P],
                            rhs=rhs[h][:, j * P:(j + 1) * P],
                            start=(j == 0), stop=(j == CPT - 1))
                    nc.tensor.matmul(
                        out=den_ps[:, 0:4],
                        lhsT=s01[:, j * P:(j + 1) * P],
                        rhs=w[:, j * 4:(j + 1) * 4],
                        start=(j == 0), stop=(j == CPT - 1))
                # epilogue
                if B_STOP < 6:
                    continue
                den = pb_ep.tile([P, 4], f32, tag='den4')
                nc.vector.tensor_scalar(out=den, in0=den_ps[:, 0:4],
                                        scalar1=1e-20, scalar2=None,
                                        op0=mybir.AluOpType.max)
                rec = pb_ep.tile([P, 4], f32, tag='rec')
                nc.vector.reciprocal(out=rec, in_=den)
                out1 = ps_ep.tile([P, H * C], f32, tag='ep')
                for h in range(H):
                    an = pb_ep.tile([P, P], bf, tag=f'an{h}')
                    nc.scalar.activation(an, pss[h][:, 0:128],
                                         mybir.ActivationFunctionType.Copy,
                                         scale=rec[:, h:h + 1])
                    psT2 = ps_sm.tile([P, P], bf, tag='psT', name='psT2')
                    nc.tensor.transpose(out=psT2, in_=an, identity=s_ident)
                    anT = pb_ep.tile([P, P], bf, tag=f'anT{h}')
                    nc.vector.tensor_copy(out=anT, in_=psT2)
                    nc.tensor.matmul(out=out1[:, h * C:(h + 1) * C],
                                     lhsT=anT,
                                     rhs=s_w1[:, h * C:(h + 1) * C],
                                     start=True, stop=True)
                zb = pb_ep.tile([P, H * C], f32, tag='zb')
                nc.vector.tensor_tensor(out=zb, in0=out1, in1=s_b1,
                                        op=mybir.AluOpType.add)
                zr = pb_ep.tile([P, H * C], f32, tag='zr')
                nc.scalar.activation(zr, zb, mybir.ActivationFunctionType.Relu)
                zm = pb_ep.tile([P, H * C], f32, tag='zm')
                nc.vector.tensor_tensor(out=zm, in0=zb, in1=zr,
                                        op=mybir.AluOpType.subtract)
                ze = pb_ep.tile([P, H * C], f32, tag='ze')
                nc.scalar.activation(ze, zm, mybir.ActivationFunctionType.Exp)
                hb = pb_ep.tile([P, H * C], bf, tag='hb')
                nc.vector.scalar_tensor_tensor(out=hb, in0=zr, scalar=-1.0,
                                               in1=ze,
                                               op0=mybir.AluOpType.add,
                                               op1=mybir.AluOpType.add)
                xw2 = ps_ep.tile([P, 42], f32, tag='ep', name='xw2')
                for kk in range(2):
                    psT3 = ps_sm.tile([P, P], bf, tag='psT', name='psT3')
                    nc.tensor.transpose(out=psT3, in_=hb[:, kk * P:(kk + 1) * P],
                                        identity=s_ident)
                    hT = pb_ep.tile([P, P], bf, tag=f'hT{kk}')
                    nc.vector.tensor_copy(out=hT, in_=psT3)
                    nc.tensor.matmul(out=xw2, lhsT=hT,
                                     rhs=s_w2e[:, kk * 42:(kk + 1) * 42],
                                     start=(kk == 0), stop=(kk == 1))
                t2r = pb_ep.tile([P, ROW], bf, tag='t2r')
                nc.vector.memset(t2r[:, 42:ROW], 0.0)
                nc.vector.tensor_copy(out=t2r[:, 0:40], in_=xw2[:, 0:40])
                nc.vector.tensor_tensor(out=t2r[:, 40:42], in0=xw2[:, 40:42],
                                        in1=s_t2bias[:, 2 * t:2 * t + 2],
                                        op=mybir.AluOpType.add)
                nc.sync.dma_start(out=t2_local[t * P:(t + 1) * P, :], in_=t2r)

        # ---------------- phase C: allgather (chunked) ----------------
        CHK = NT // AG_CHUNKS if NT % AG_CHUNKS == 0 else NT
        nch = NT // CHK
        for a in range(nch if 'C' in phases else 0):
            nc.gpsimd.collective_compute(
                'AllGather', mybir.AluOpType.bypass,
                ins=[t2_local[a * CHK * P:(a + 1) * CHK * P, :]],
                outs=[t2_full.rearrange('(c r) e -> c r e', c=NC)
                      [:, a * CHK * P:(a + 1) * CHK * P, :]],
                replica_groups=[list(range(NC))])

        # ---------------- phase D: layer-2 edges + log_softmax ------------
        with tc.tile_pool(name='pd', bufs=2) as pd, \
             tc.tile_pool(name='pd_rhs', bufs=2) as pd_rhs, \
             tc.tile_pool(name='pd_ep', bufs=2) as pd_ep, \
             tc.tile_pool(name='ps2', bufs=2, space='PSUM') as ps2, \
             tc.tile_pool(name='ps2_sm', bufs=2, space='PSUM') as ps2_sm:
            for t in range(NT if 'D' in phases else 0):
                c0 = t * CPT
                g2 = pd.tile([P, CPT, ROW], bf, tag='g2')
                nc.gpsimd.dma_gather(g2[:, 0:CA, :], t2_full[:, :],
                                     s_idxA2[:, t * CA * 8:(t + 1) * CA * 8],
                                     CA * P, CA * P, ROW, single_packet=False)
                nc.gpsimd.dma_gather(g2[:, CA:CPT, :], t2_full[RB:nrow2, :],
                                     s_idxB2[:, t * CB * 8:(t + 1) * CB * 8],
                                     CB * P, CB * P, ROW, single_packet=False)
                ad2 = pd.tile([P, 2], bf, tag='ad2')
                nc.sync.dma_start(out=ad2,
                                  in_=t2_local[t * P:(t + 1) * P, 40:42])
                s01 = pd.tile([P, CPT * P], bf, tag='s01b')
                nc.vector.tensor_tensor(
                    out=s01.rearrange('p (k j) -> p k j', j=P),
                    in0=s_iota.rearrange('p (k j) -> p k j', j=P),
                    in1=s_dstloc[:, c0:c0 + CPT].to_broadcast([P, CPT, P]),
                    op=mybir.AluOpType.is_equal)
                s01Tt = pd_rhs.tile([P, CPT * P], bf, tag='s01Tt2')
                nc.sync.dma_start(
                    out=s01Tt,
                    in_=m_s01T[:, t * CPT * P:(t + 1) * CPT * P])
                sstage = pd.tile([P, CPT], bf, tag='sst2')
                acc = ps2.tile([P, 40], f32, tag='acc2')
                den_ps = ps2.tile([P, 8], f32, tag='den2p')
                sc2_ps = ps2_sm.tile([P, CPT], f32, tag='psT2', name='sc2ps')
                for j in range(CPT):
                    nc.tensor.matmul(out=sc2_ps[:, j:j + 1],
                                     lhsT=s01Tt[:, j * P:(j + 1) * P],
                                     rhs=ad2[:, 1:2], start=True, stop=True)
                nc.vector.tensor_tensor(
                    out=sstage.rearrange('p (k e) -> p k e', e=1),
                    in0=g2[:, :, 40:41],
                    in1=sc2_ps.rearrange('p (k e) -> p k e', e=1),
                    op=mybir.AluOpType.add)
                st = pd.tile([P, CPT], bf, tag='st2')
                nc.vector.tensor_scalar(out=st, in0=sstage, scalar1=NEG_SLOPE,
                                        scalar2=None, op0=mybir.AluOpType.mult)
                sl = pd.tile([P, CPT], bf, tag='sl2')
                nc.vector.tensor_tensor(out=sl, in0=st, in1=sstage,
                                        op=mybir.AluOpType.max)
                w = pd.tile([P, CPT], bf, tag='w2')
                nc.scalar.activation(w, sl, mybir.ActivationFunctionType.Exp)
                r = pd_rhs.tile([P, CPT * 40], bf, tag='rhs2')
                nc.vector.tensor_tensor(
                    out=r.rearrange('p (k j) -> p k j', j=40),
                    in0=g2[:, :, 0:40],
                    in1=w.rearrange('p (k o) -> p k o', o=1)
                    .to_broadcast([P, CPT, 40]),
                    op=mybir.AluOpType.mult)
                for j in range(CPT):
                    nc.tensor.matmul(out=acc[:, :],
                                     lhsT=s01[:, j * P:(j + 1) * P],
                                     rhs=r[:, j * 40:(j + 1) * 40],
                                     start=(j == 0), stop=(j == CPT - 1))
                    nc.tensor.matmul(out=den_ps[:, 0:1],
                                     lhsT=s01[:, j * P:(j + 1) * P],
                                     rhs=w[:, j:j + 1],
                                     start=(j == 0), stop=(j == CPT - 1))
                den = pd_ep.tile([P, 1], f32, tag='den2')
                nc.vector.tensor_scalar(out=den, in0=den_ps[:, 0:1],
                                        scalar1=1e-20, scalar2=None,
                                        op0=mybir.AluOpType.max)
                rec = pd_ep.tile([P, 1], f32, tag='rec2')
                nc.vector.reciprocal(out=rec, in_=den)
                o = pd_ep.tile([P, 40], f32, tag='o')
                nc.scalar.activation(o, acc[:, 0:40],
                                     mybir.ActivationFunctionType.Copy,
                                     scale=rec)
                ob = pd_ep.tile([P, 40], f32, tag='ob')
                nc.vector.tensor_tensor(out=ob, in0=o, in1=s_b2,
                                        op=mybir.AluOpType.add)
                ex = pd_ep.tile([P, 40], f32, tag='ex')
                sm = pd_ep.tile([P, 1], f32, tag='sm')
                nc.scalar.activation(ex, ob, mybir.ActivationFunctionType.Exp,
                                     accum_out=sm)
                rsm = pd_ep.tile([P, 1], f32, tag='rsm')
                nc.vector.reciprocal(out=rsm, in_=sm)
                nlg = pd_ep.tile([P, 1], f32, tag='nlg')
                nc.scalar.activation(nlg, rsm,
                                     mybir.ActivationFunctionType.Ln)
                fin = pd_ep.tile([P, 40], f32, tag='fin')
                nc.scalar.activation(fin, ob,
                                     mybir.ActivationFunctionType.Identity,
                                     bias=nlg)
                nc.sync.dma_start(out=out_d[t * P:(t + 1) * P, :], in_=fin)

    nc.compile()
    return nc


# ----------------------------------------------------------------- entry

_CACHE = {}


def prepare(x, edge_index, W1, att_src1, att_dst1, b1, W2, att_src2, att_dst2,
            b2, build=True, debug=False):
    x = np.asarray(x, F32)
    edge_index = np.asarray(edge_index)
    n_nodes = x.shape[0]

    meta, shapes, nstart, ncnt = _prep(edge_index, n_nodes)
    tables = _host_tables(x, np.asarray(W1, F32), np.asarray(att_src1, F32),
                          np.asarray(att_dst1, F32), np.asarray(W2, F32),
                          np.asarray(att_src2, F32), np.asarray(att_dst2, F32),
                          shapes)
    nc = None
    if build:
        key = (shapes['NT'], shapes['THR'], shapes['RB'], n_nodes, debug)
        if key not in _CACHE:
            _CACHE[key] = _build(shapes, n_nodes, debug=debug)
        nc = _CACHE[key]

    b1bc = np.broadcast_to(np.asarray(b1, F32), (P, H * C)).copy()
    b2bc = np.broadcast_to(np.asarray(b2, F32), (P, 40)).copy()

    in_maps = []
    for c in range(NC):
        in_maps.append(dict(
            t1x=tables['t1x'], xT=tables['xT'], wsd=tables['wsd'],
            w1=tables['w1'], w2e=tables['w2e'], iota_k=tables['iota_k'],
            ident=tables['ident'], b1bc=b1bc, b2bc=b2bc,
            t2bias=meta['t2bias'][c],
            idxA1=meta['idxA1'][c], idxB1=meta['idxB1'][c],
            idxA2=meta['idxA2'][c], idxB2=meta['idxB2'][c],
            dstloc=meta['dstloc'][c], adoffs=meta['adoffs'][c],
            s01T=meta['s01T'][c],
        ))
    return dict(nc=nc, in_maps=in_maps, shapes=shapes, nstart=nstart,
                ncnt=ncnt, n_nodes=n_nodes)


def assemble(ctx_run, outs):
    NT = ctx_run['shapes']['NT']
    nstart, ncnt = ctx_run['nstart'], ctx_run['ncnt']
    out = np.zeros((ctx_run['n_nodes'], 40), F32)
    for c in range(NC):
        oc = outs[c]['out']
        for t in range(NT):
            cnt = int(ncnt[c, t])
            if cnt == 0:
                continue
            n0 = int(nstart[c, t])
            out[n0:n0 + cnt] = oc[t * P:t * P + cnt]
    return out


def kernel(x, edge_index, W1, att_src1, att_dst1, b1, W2, att_src2, att_dst2, b2):
    ctx_run = prepare(x, edge_index, W1, att_src1, att_dst1, b1,
                      W2, att_src2, att_dst2, b2)
    res = run_bass_kernel_spmd(ctx_run['nc'], ctx_run['in_maps'],
                               list(range(NC)))
    return assemble(ctx_run, res.results)


# revision 34
# speedup vs baseline: 1.6788x; 1.0142x over previous
"""Trainium2 Bass kernel for 2-layer GAT (nn_GAT_90460601188538).

Strategy: edges sorted by destination; destination nodes greedily packed
into 128-slot tiles; tiles split contiguously across 8 cores. Per
edge-chunk of 128, a one-hot selection matrix (iota == dst_slot) turns the
segmented softmax-sum and scatter-add into PE matmuls accumulating in
PSUM. Softmax runs without max subtraction (scores are O(1)) as
unnormalized sums plus one divide per node. Edge-source features are
fetched with dma_gather (int16 indices), so every table is split at one
global node threshold THR into two halves gathered separately; each tile's
chunks are partitioned into group A (src < THR) and group B. Pad edge
slots point at row 0 with dst slot 200, whose one-hot row is all zero, so
they contribute nothing. Layer-2 features are exchanged with an AllGather.
"""
import sys
sys.path.insert(0, '/opt/trn_rl_repo')
from contextlib import ExitStack

import numpy as np
import ml_dtypes

import concourse.bacc as bacc
import concourse.tile as tile
from concourse import bass, mybir, library_config
from concourse.bass_utils import run_bass_kernel_spmd

BF16 = ml_dtypes.bfloat16
F32 = np.float32

P = 128
NC = 8
CA = 12               # group-A chunks per tile (src < THR)
CB = 8                # group-B chunks per tile (src >= THR)
CPT = CA + CB
NEG_SLOPE = 0.2
NEG_BIG = -10000.0
ROW = 128             # table row elems (bf16) = 256B
H, C, F = 4, 64, 128
THR_CAP = 30720       # node split threshold cap (int16 table indexing)
B_STOP = 6            # debug: truncate phase-B body (1..6)


# ----------------------------------------------------------------- host prep

def _wrap_idx(flat):
    """[n] int -> dma_gather layout [128, n//16] int16 (16-wrap, replicated)."""
    n = flat.shape[0]
    out = np.zeros((P, n // 16), np.int16)
    cols = flat.reshape(n // 16, 16).T.astype(np.int16)   # [16, n//16]
    for rep in range(8):
        out[rep * 16:(rep + 1) * 16, :] = cols
    return out


def _prep(edge_index, n_nodes):
    src = np.concatenate([edge_index[0], np.arange(n_nodes, dtype=np.int64)])
    dst = np.concatenate([edge_index[1], np.arange(n_nodes, dtype=np.int64)])
    perm = np.argsort(dst, kind='stable')
    src_s = src[perm].astype(np.int64)
    dst_s = dst[perm].astype(np.int64)
    deg = np.bincount(dst_s, minlength=n_nodes)

    THR = min(THR_CAP, n_nodes)   # node split threshold (A: src < THR)
    capA, capB = CA * P, CB * P

    # greedy pack consecutive nodes: <=P nodes, <=capA A-edges, <=capB B-edges
    isB = (src_s >= THR)
    degA = np.bincount(dst_s[~isB], minlength=n_nodes)
    degB = deg - degA
    cumA = np.concatenate([[0], np.cumsum(degA)])
    cumB = np.concatenate([[0], np.cumsum(degB)])
    tiles = []
    n = 0
    while n < n_nodes:
        hiA = int(np.searchsorted(cumA, cumA[n] + capA, side='right')) - 1 - n
        hiB = int(np.searchsorted(cumB, cumB[n] + capB, side='right')) - 1 - n
        cnt = max(1, min(hiA, hiB, P, n_nodes - n))
        assert cumA[n + cnt] - cumA[n] <= capA
        assert cumB[n + cnt] - cumB[n] <= capB
        tiles.append((n, cnt))
        n += cnt
    T = len(tiles)
    NT = (T + 1 + NC - 1) // NC      # >=1 pad tile overall
    NCH = NT * CPT

    nrow1 = ((n_nodes + P - 1) // P) * P
    nrow2 = NC * NT * P

    # node -> (global slot row, tile)
    node_row = np.zeros(n_nodes, np.int64)
    nstart = np.zeros((NC, NT), np.int64)
    ncnt = np.zeros((NC, NT), np.int64)
    for gi, (n0, cnt) in enumerate(tiles):
        c, t = gi // NT, gi % NT
        nstart[c, t] = n0
        ncnt[c, t] = cnt
        node_row[n0:n0 + cnt] = gi * P + np.arange(cnt)
    RB = int(node_row[THR]) if THR < n_nodes else max(0, nrow2 - P)
    assert RB < 32768 and nrow2 - RB <= 32768, (RB, nrow2)
    assert nrow1 - THR <= 32768, (THR, nrow1)

    # per-core streams
    idxA1 = np.zeros((NC, P, NT * CA * 8), np.int16)
    idxB1 = np.zeros((NC, P, NT * CB * 8), np.int16)
    idxA2 = np.zeros((NC, P, NT * CA * 8), np.int16)
    idxB2 = np.zeros((NC, P, NT * CB * 8), np.int16)
    dstloc = np.full((NC, P, NCH), BF16(200.0), BF16)
    adoffs = np.zeros((NC, P, NT), np.int32)      # slot node ids (a_d tile)
    t2bias = np.full((NC, P, NT * 2), NEG_BIG, F32)

    # edge ranges per tile
    epos = 0
    edge_of_tile = []
    for (n0, cnt) in tiles:
        e0 = epos
        ecnt = int(deg[n0:n0 + cnt].sum())
        edge_of_tile.append((e0, ecnt))
        epos += ecnt

    for gi, (n0, cnt) in enumerate(tiles):
        c, t = gi // NT, gi % NT
        e0, ecnt = edge_of_tile[gi]
        es = src_s[e0:e0 + ecnt]
        ed = dst_s[e0:e0 + ecnt]
        sl = (ed - n0).astype(np.int64)
        selB = es >= THR
        esA, slA = es[~selB], sl[~selB]
        esB, slB = es[selB], sl[selB]
        fa = np.zeros(capA, np.int64)
        fa[:len(esA)] = esA
        fb = np.zeros(capB, np.int64)
        fb[:len(esB)] = esB - THR
        idxA1[c, :, t * CA * 8:(t + 1) * CA * 8] = _wrap_idx(fa)
        idxB1[c, :, t * CB * 8:(t + 1) * CB * 8] = _wrap_idx(fb)
        fa2 = np.zeros(capA, np.int64)
        fa2[:len(esA)] = node_row[esA]
        fb2 = np.zeros(capB, np.int64)
        fb2[:len(esB)] = node_row[esB] - RB
        idxA2[c, :, t * CA * 8:(t + 1) * CA * 8] = _wrap_idx(fa2)
        idxB2[c, :, t * CB * 8:(t + 1) * CB * 8] = _wrap_idx(fb2)
        # dstloc: chunks 0..CA-1 = A slots, CA..CPT-1 = B slots
        dl = np.full((CPT, P), 200.0, np.float64)
        ia = np.arange(len(esA))
        dl[ia // P, ia % P] = slA
        ib = np.arange(len(esB))
        dl[CA + ib // P, ib % P] = slB
        dstloc[c, :, t * CPT:(t + 1) * CPT] = dl.T.astype(BF16)
        ad = np.full(P, n_nodes, np.int64)    # pad slots -> dummy row
        ad[:cnt] = n0 + np.arange(cnt)
        adoffs[c, :, t] = ad
        t2bias[c, :, 2 * t:2 * t + 2] = 0.0

    # host-built transposed one-hot: s01T[d, (t,j,e)] = 1 iff dst slot of
    # edge slot (t,j,e) == d; lhsT for broadcasting a_d to edge slots.
    dvals = np.arange(P, dtype=np.float64)
    s01T = np.zeros((NC, P, NT * CPT * P), BF16)
    s01h = np.zeros((NC, P, NT * CPT * P), BF16)
    for c in range(NC):
        # dstloc[c] is [P(e), NCH(t,j)] -> oh [d, t*CPT+j, e]
        oh = (dstloc[c].astype(np.float64).T[None, :, :] == dvals[:, None, None])
        s01T[c] = np.ascontiguousarray(oh).reshape(P, -1).astype(BF16)
        # s01h[e, (t,j,d)] = oh[d, tj, e]
        s01h[c] = np.ascontiguousarray(oh.transpose(2, 1, 0)).reshape(P, -1).astype(BF16)

    meta = dict(idxA1=idxA1, idxB1=idxB1, idxA2=idxA2, idxB2=idxB2,
                dstloc=dstloc, adoffs=adoffs, t2bias=t2bias, s01T=s01T,
                s01h=s01h)
    shapes = dict(T=T, NT=NT, NCH=NCH, nrow1=nrow1, nrow2=nrow2,
                  THR=THR, RB=RB)
    return meta, shapes, nstart, ncnt


def _host_tables(x, W1, att_src1, att_dst1, W2, att_src2, att_dst2, shapes):
    n_nodes = x.shape[0]
    nrow1 = shapes['nrow1']

    t1x = np.zeros((nrow1, 2 * ROW), BF16)
    t1x[:n_nodes, :F] = x.astype(BF16)

    xT = np.zeros((P, nrow1), BF16)
    xT[:, :n_nodes] = x.astype(BF16).T

    W1r = W1.reshape(F, H, C)
    wsd = np.zeros((P, 8), BF16)
    wsd[:, 0:4] = np.einsum('fhc,hc->fh', W1r, att_src1).astype(BF16)
    wsd[:, 4:8] = np.einsum('fhc,hc->fh', W1r, att_dst1).astype(BF16)

    w1 = W1.astype(BF16)                                   # [128, 256]
    ws2 = (W2 @ att_src2[0])[:, None]
    wd2 = (W2 @ att_dst2[0])[:, None]
    w2e = np.concatenate([W2, ws2, wd2], axis=1).astype(BF16)  # [256, 42]
    w2e_packed = np.concatenate([w2e[0:P], w2e[P:2 * P]], axis=1)  # [128, 84]

    iota_k = np.tile(np.arange(P, dtype=BF16), CPT)[None, :].repeat(P, 0)
    ident = np.eye(P, dtype=BF16)
    return dict(t1x=t1x, xT=xT, wsd=wsd, w1=w1, w2e=w2e_packed,
                iota_k=np.ascontiguousarray(iota_k), ident=ident)


# ------------------------------------------------------------- device program

def _build(shapes, n_nodes, debug=False, phases='ABCD'):
    B_STOP = globals()['B_STOP']
    NT, NCH = shapes['NT'], shapes['NCH']
    nrow1, nrow2 = shapes['nrow1'], shapes['nrow2']
    THR, RB = shapes['THR'], shapes['RB']
    NAT = nrow1
    bf = mybir.dt.bfloat16
    f32 = mybir.dt.float32
    i32 = mybir.dt.int32
    i16 = mybir.dt.int16
    AG_CHUNKS = 4
    assert NT % AG_CHUNKS == 0 or True

    nc = bacc.Bacc('TRN2', target_bir_lowering=False, debug=False,
                   num_devices=NC)

    def inp(name, shape, dt):
        return nc.dram_tensor(name, list(shape), dt, kind='ExternalInput').ap()

    t1x = inp('t1x', (nrow1, 2 * ROW), bf)
    xT = inp('xT', (P, nrow1), bf)
    wsd = inp('wsd', (P, 8), bf)
    w1 = inp('w1', (P, H * C), bf)
    w2e = inp('w2e', (P, 2 * 42), bf)
    iota_k = inp('iota_k', (P, CPT * P), bf)
    ident = inp('ident', (P, P), bf)
    b1bc = inp('b1bc', (P, H * C), f32)
    b2bc = inp('b2bc', (P, 40), f32)
    t2bias = inp('t2bias', (P, NT * 2), f32)
    m_idxA1 = inp('idxA1', (P, NT * CA * 8), i16)
    m_idxB1 = inp('idxB1', (P, NT * CB * 8), i16)
    m_idxA2 = inp('idxA2', (P, NT * CA * 8), i16)
    m_idxB2 = inp('idxB2', (P, NT * CB * 8), i16)
    m_dstloc = inp('dstloc', (P, NCH), bf)
    m_adoffs = inp('adoffs', (P, NT), i32)
    m_s01T = inp('s01T', (P, NT * CPT * P), bf)
    m_s01h = inp('s01h', (P, NT * CPT * P), bf)

    out_d = nc.dram_tensor('out', [NT * P, 40], f32, kind='ExternalOutput').ap()

    adtab = nc.dram_tensor('adtab', [NAT, 4], bf).ap()     # [a_d(4)]

    dbg = {}

    with tile.TileContext(nc) as tc, ExitStack() as ctx:
        nc.gpsimd.load_library(library_config.mlp)
        dram = ctx.enter_context(tc.tile_pool(name='dram', bufs=1, space='DRAM'))
        t2_local = dram.tile([NT * P, ROW], bf)
        t2_full = dram.tile([nrow2, ROW], bf, addr_space='Shared')

        consts = ctx.enter_context(tc.tile_pool(name='consts', bufs=1))
        meta = ctx.enter_context(tc.tile_pool(name='meta', bufs=1))

        s_wsd = consts.tile([P, 8], bf)
        nc.sync.dma_start(out=s_wsd, in_=wsd)
        s_w1 = consts.tile([P, H * C], bf)
        nc.sync.dma_start(out=s_w1, in_=w1)
        s_w2e = consts.tile([P, 2 * 42], bf)
        nc.sync.dma_start(out=s_w2e, in_=w2e)
        s_iota = consts.tile([P, CPT * P], bf)
        nc.sync.dma_start(out=s_iota, in_=iota_k)
        s_ident = consts.tile([P, P], bf)
        nc.sync.dma_start(out=s_ident, in_=ident)
        s_b1 = consts.tile([P, H * C], f32)
        nc.sync.dma_start(out=s_b1, in_=b1bc)
        s_b2 = consts.tile([P, 40], f32)
        nc.sync.dma_start(out=s_b2, in_=b2bc)
        s_t2bias = consts.tile([P, NT * 2], f32)
        nc.sync.dma_start(out=s_t2bias, in_=t2bias)

        s_idxA1 = meta.tile([P, NT * CA * 8], i16)
        nc.sync.dma_start(out=s_idxA1, in_=m_idxA1)
        s_idxB1 = meta.tile([P, NT * CB * 8], i16)
        nc.sync.dma_start(out=s_idxB1, in_=m_idxB1)
        s_idxA2 = meta.tile([P, NT * CA * 8], i16)
        nc.sync.dma_start(out=s_idxA2, in_=m_idxA2)
        s_idxB2 = meta.tile([P, NT * CB * 8], i16)
        nc.sync.dma_start(out=s_idxB2, in_=m_idxB2)
        s_dstloc = meta.tile([P, NCH], bf)
        nc.sync.dma_start(out=s_dstloc, in_=m_dstloc)
        s_adoffs = meta.tile([P, NT], i32)
        nc.sync.dma_start(out=s_adoffs, in_=m_adoffs)

        # ---------------- phase A: a_s (astab) and a_d (adtab) tables ------
        GA = 8
        n_a_tiles = nrow1 // P
        with tc.tile_pool(name='pa', bufs=2) as pa, \
             tc.tile_pool(name='pa_ps', bufs=2, space='PSUM') as pa_ps:
            for t0 in range(0, n_a_tiles if 'A' in phases else 0, GA):
                g = min(GA, n_a_tiles - t0)
                xt = pa.tile([P, GA * P], bf, tag='xt')
                nc.sync.dma_start(out=xt[:, :g * P],
                                  in_=xT[:, t0 * P:(t0 + g) * P])
                ps = pa_ps.tile([P, GA * 8], f32, tag='ps')
                for j in range(g):
                    nc.tensor.matmul(out=ps[:, j * 8:(j + 1) * 8],
                                     lhsT=xt[:, j * P:(j + 1) * P],
                                     rhs=s_wsd, start=True, stop=True)
                sa = pa.tile([P, GA * 4], bf, tag='sa')
                nc.vector.tensor_copy(
                    out=sa[:, :g * 4].rearrange('p (j e) -> p j e', e=4),
                    in_=ps[:, :g * 8].rearrange('p (j e) -> p j e', e=8)[:, :, 0:4])
                sd = pa.tile([P, GA * 4], bf, tag='sd')
                nc.vector.tensor_copy(
                    out=sd[:, :g * 4].rearrange('p (j e) -> p j e', e=4),
                    in_=ps[:, :g * 8].rearrange('p (j e) -> p j e', e=8)[:, :, 4:8])
                as_ap = bass.AP(tensor=t1x.tensor,
                                offset=t0 * P * 2 * ROW + F,
                                ap=[[2 * ROW, P], [P * 2 * ROW, g], [1, 4]])
                nc.sync.dma_start(
                    out=as_ap,
                    in_=sa[:, :g * 4].rearrange('p (j e) -> p j e', e=4))
                ad_ap = bass.AP(tensor=adtab.tensor,
                                offset=t0 * P * 4,
                                ap=[[4, P], [P * 4, g], [1, 4]])
                nc.sync.dma_start(
                    out=ad_ap,
                    in_=sd[:, :g * 4].rearrange('p (j e) -> p j e', e=4))

        if 'Z' in phases:   # minimal: write zeros to out
            with tc.tile_pool(name='pz', bufs=1) as pz:
                zt = pz.tile([P, 40], f32)
                nc.vector.memset(zt, 0.0)
                for t in range(NT):
                    nc.sync.dma_start(out=out_d[t * P:(t + 1) * P, :], in_=zt)

        # ---------------- phase B: layer-1 edges + tile epilogue -----------
        with tc.tile_pool(name='pb', bufs=2) as pb, \
             tc.tile_pool(name='pb_rhs', bufs=2) as pb_rhs, \
             tc.tile_pool(name='pb_ep', bufs=2) as pb_ep, \
             tc.tile_pool(name='ps_acc', bufs=1, space='PSUM') as ps_acc, \
             tc.tile_pool(name='ps_sm', bufs=2, space='PSUM') as ps_sm, \
             tc.tile_pool(name='ps_ep', bufs=1, space='PSUM') as ps_ep:
            for t in range(NT if 'B' in phases else 0):
                c0 = t * CPT
                gx = pb.tile([P, CPT, 2 * ROW], bf, tag='gx')
                nc.gpsimd.dma_gather(gx[:, 0:CA, :], t1x,
                                     s_idxA1[:, t * CA * 8:(t + 1) * CA * 8],
                                     CA * P, CA * P, 2 * ROW,
                                     single_packet=False)
                nc.gpsimd.dma_gather(gx[:, CA:CPT, :], t1x[THR:nrow1, :],
                                     s_idxB1[:, t * CB * 8:(t + 1) * CB * 8],
                                     CB * P, CB * P, 2 * ROW,
                                     single_packet=False)
                adt = pb.tile([P, 4], bf, tag='adt')
                nc.gpsimd.indirect_dma_start(
                    out=adt, out_offset=None, in_=adtab,
                    in_offset=bass.IndirectOffsetOnAxis(
                        ap=s_adoffs[:, t:t + 1], axis=0))
                if B_STOP < 2:
                    continue
                # one-hot S01 for the whole tile (host-built)
                s01 = pb.tile([P, CPT * P], bf, tag='s01')
                nc.sync.dma_start(
                    out=s01, in_=m_s01h[:, t * CPT * P:(t + 1) * CPT * P])
                # scores: a_d broadcast via host s01T matmuls into PSUM,
                # then one strided add of the gathered a_s strip
                s01Tt = pb_rhs.tile([P, CPT * P], bf, tag='s01Tt')
                nc.sync.dma_start(
                    out=s01Tt,
                    in_=m_s01T[:, t * CPT * P:(t + 1) * CPT * P])
                sstage = pb.tile([P, CPT * 4], bf, tag='sstage')
                pss = [ps_acc.tile([P, P], f32, tag=f'ph{h}',
                                   name=f'ph{h}')
                       for h in range(H)]
                den_ps = ps_acc.tile([P, 8], f32, tag='den')
                w = pb.tile([P, CPT * 4], bf, tag='w')
                if B_STOP < 3:
                    continue
                sc_ps = ps_sm.tile([P, CPT * 4], f32, tag='psT', name='scps')
                for j in range(CPT):
                    nc.tensor.matmul(out=sc_ps[:, j * 4:(j + 1) * 4],
                                     lhsT=s01Tt[:, j * P:(j + 1) * P],
                                     rhs=adt, start=True, stop=True)
                nc.vector.tensor_tensor(
                    out=sstage.rearrange('p (k e) -> p k e', e=4),
                    in0=gx[:, :, F:F + 4],
                    in1=sc_ps.rearrange('p (k e) -> p k e', e=4),
                    op=mybir.AluOpType.add)
                if B_STOP < 4:
                    continue
                st = pb.tile([P, CPT * 4], bf, tag='st')
                nc.vector.tensor_scalar(out=st, in0=sstage, scalar1=NEG_SLOPE,
                                        scalar2=None, op0=mybir.AluOpType.mult)
                sl = pb.tile([P, CPT * 4], bf, tag='sl')
                nc.vector.tensor_tensor(out=sl, in0=st, in1=sstage,
                                        op=mybir.AluOpType.max)
                nc.scalar.activation(w, sl, mybir.ActivationFunctionType.Exp)
                # rhs_h = w_h * x ; accumulate
                rhs = []
                w3 = w.rearrange('p (k e) -> p k e', e=4)
                for h in range(H):
                    r = pb_rhs.tile([P, CPT * P], bf, tag=f'rhs{h}')
                    nc.vector.tensor_tensor(
                        out=r.rearrange('p (k j) -> p k j', j=P),
                        in0=gx[:, :, 0:F],
                        in1=w3[:, :, h:h + 1].to_broadcast([P, CPT, P]),
                        op=mybir.AluOpType.mult)
                    rhs.append(r)
                if B_STOP < 5:
                    continue
                for j in range(CPT):
                    for h in range(H):
                        nc.tensor.matmul(
                            out=pss[h][:, 0:128],
                            lhsT=rhs[h][:, j * P:(j + 1) * P],
                            rhs=s01[:, j * P:(j + 1) * P],
                            start=(j == 0), stop=(j == CPT - 1))
                    nc.tensor.matmul(
                        out=den_ps[:, 0:4],
                        lhsT=s01[:, j * P:(j + 1) * P],
                        rhs=w[:, j * 4:(j + 1) * 4],
                        start=(j == 0), stop=(j == CPT - 1))
                # epilogue
                if B_STOP < 6:
                    continue
                den = pb_ep.tile([P, 4], f32, tag='den4')
                nc.vector.tensor_scalar(out=den, in0=den_ps[:, 0:4],
                                        scalar1=1e-20, scalar2=None,
                                        op0=mybir.AluOpType.max)
                rec = pb_ep.tile([P, 4], f32, tag='rec')
                nc.vector.reciprocal(out=rec, in_=den)
                out1 = ps_ep.tile([P, H * C], f32, tag='ep')
                for h in range(H):
                    asb = pb_ep.tile([P, P], bf, tag=f'an{h}')
                    nc.scalar.activation(asb, pss[h][:, 0:128],
                                         mybir.ActivationFunctionType.Copy)
                    nc.tensor.matmul(out=out1[:, h * C:(h + 1) * C],
                                     lhsT=asb,
                                     rhs=s_w1[:, h * C:(h + 1) * C],
                                     start=True, stop=True)
                zsc = pb_ep.tile([P, H * C], f32, tag='zsc')
                for h in range(H):
                    nc.scalar.activation(zsc[:, C * h:C * (h + 1)],
                                         out1[:, C * h:C * (h + 1)],
                                         mybir.ActivationFunctionType.Copy,
                                         scale=rec[:, h:h + 1])
                zb = pb_ep.tile([P, H * C], f32, tag='zb')
                nc.vector.tensor_tensor(out=zb, in0=zsc, in1=s_b1,
                                        op=mybir.AluOpType.add)
                zr = pb_ep.tile([P, H * C], f32, tag='zr')
                nc.scalar.activation(zr, zb, mybir.ActivationFunctionType.Relu)
                zm = pb_ep.tile([P, H * C], f32, tag='zm')
                nc.vector.tensor_tensor(out=zm, in0=zb, in1=zr,
                                        op=mybir.AluOpType.subtract)
                ze = pb_ep.tile([P, H * C], f32, tag='ze')
                nc.scalar.activation(ze, zm, mybir.ActivationFunctionType.Exp)
                hb = pb_ep.tile([P, H * C], bf, tag='hb')
                nc.vector.scalar_tensor_tensor(out=hb, in0=zr, scalar=-1.0,
                                               in1=ze,
                                               op0=mybir.AluOpType.add,
                                               op1=mybir.AluOpType.add)
                xw2 = ps_ep.tile([P, 42], f32, tag='ep', name='xw2')
                for kk in range(2):
                    psT3 = ps_sm.tile([P, P], bf, tag='psT', name='psT3')
                    nc.tensor.transpose(out=psT3, in_=hb[:, kk * P:(kk + 1) * P],
                                        identity=s_ident)
                    hT = pb_ep.tile([P, P], bf, tag=f'hT{kk}')
                    nc.vector.tensor_copy(out=hT, in_=psT3)
                    nc.tensor.matmul(out=xw2, lhsT=hT,
                                     rhs=s_w2e[:, kk * 42:(kk + 1) * 42],
                                     start=(kk == 0), stop=(kk == 1))
                t2r = pb_ep.tile([P, ROW], bf, tag='t2r')
                nc.vector.memset(t2r[:, 42:ROW], 0.0)
                nc.vector.tensor_copy(out=t2r[:, 0:40], in_=xw2[:, 0:40])
                nc.vector.tensor_tensor(out=t2r[:, 40:42], in0=xw2[:, 40:42],
                                        in1=s_t2bias[:, 2 * t:2 * t + 2],
                                        op=mybir.AluOpType.add)
                nc.sync.dma_start(out=t2_local[t * P:(t + 1) * P, :], in_=t2r)

        # ---------------- phase C: allgather (chunked) ----------------
        CHK = NT // AG_CHUNKS if NT % AG_CHUNKS == 0 else NT
        nch = NT // CHK
        for a in range(nch if 'C' in phases else 0):
            nc.gpsimd.collective_compute(
                'AllGather', mybir.AluOpType.bypass,
                ins=[t2_local[a * CHK * P:(a + 1) * CHK * P, :]],
                outs=[t2_full.rearrange('(c r) e -> c r e', c=NC)
                      [:, a * CHK * P:(a + 1) * CHK * P, :]],
                replica_groups=[list(range(NC))])

        # ---------------- phase D: layer-2 edges + log_softmax ------------
        with tc.tile_pool(name='pd', bufs=2) as pd, \
             tc.tile_pool(name='pd_rhs', bufs=2) as pd_rhs, \
             tc.tile_pool(name='pd_ep', bufs=2) as pd_ep, \
             tc.tile_pool(name='ps2', bufs=2, space='PSUM') as ps2, \
             tc.tile_pool(name='ps2_sm', bufs=2, space='PSUM') as ps2_sm:
            for t in range(NT if 'D' in phases else 0):
                c0 = t * CPT
                g2 = pd.tile([P, CPT, ROW], bf, tag='g2')
                nc.gpsimd.dma_gather(g2[:, 0:CA, :], t2_full[:, :],
                                     s_idxA2[:, t * CA * 8:(t + 1) * CA * 8],
                                     CA * P, CA * P, ROW, single_packet=False)
                nc.gpsimd.dma_gather(g2[:, CA:CPT, :], t2_full[RB:nrow2, :],
                                     s_idxB2[:, t * CB * 8:(t + 1) * CB * 8],
                                     CB * P, CB * P, ROW, single_packet=False)
                ad2 = pd.tile([P, 2], bf, tag='ad2')
                nc.sync.dma_start(out=ad2,
                                  in_=t2_local[t * P:(t + 1) * P, 40:42])
                s01 = pd.tile([P, CPT * P], bf, tag='s01b')
                nc.sync.dma_start(
                    out=s01, in_=m_s01h[:, t * CPT * P:(t + 1) * CPT * P])
                s01Tt = pd_rhs.tile([P, CPT * P], bf, tag='s01Tt2')
                nc.sync.dma_start(
                    out=s01Tt,
                    in_=m_s01T[:, t * CPT * P:(t + 1) * CPT * P])
                sstage = pd.tile([P, CPT], bf, tag='sst2')
                acc = ps2.tile([P, 40], f32, tag='acc2')
                den_ps = ps2.tile([P, 8], f32, tag='den2p')
                sc2_ps = ps2_sm.tile([P, CPT], f32, tag='psT2', name='sc2ps')
                for j in range(CPT):
                    nc.tensor.matmul(out=sc2_ps[:, j:j + 1],
                                     lhsT=s01Tt[:, j * P:(j + 1) * P],
                                     rhs=ad2[:, 1:2], start=True, stop=True)
                nc.vector.tensor_tensor(
                    out=sstage.rearrange('p (k e) -> p k e', e=1),
                    in0=g2[:, :, 40:41],
                    in1=sc2_ps.rearrange('p (k e) -> p k e', e=1),
                    op=mybir.AluOpType.add)
                st = pd.tile([P, CPT], bf, tag='st2')
                nc.vector.tensor_scalar(out=st, in0=sstage, scalar1=NEG_SLOPE,
                                        scalar2=None, op0=mybir.AluOpType.mult)
                sl = pd.tile([P, CPT], bf, tag='sl2')
                nc.vector.tensor_tensor(out=sl, in0=st, in1=sstage,
                                        op=mybir.AluOpType.max)
                w = pd.tile([P, CPT], bf, tag='w2')
                nc.scalar.activation(w, sl, mybir.ActivationFunctionType.Exp)
                r = pd_rhs.tile([P, CPT * 40], bf, tag='rhs2')
                nc.vector.tensor_tensor(
                    out=r.rearrange('p (k j) -> p k j', j=40),
                    in0=g2[:, :, 0:40],
                    in1=w.rearrange('p (k o) -> p k o', o=1)
                    .to_broadcast([P, CPT, 40]),
                    op=mybir.AluOpType.mult)
                for j in range(CPT):
                    nc.tensor.matmul(out=acc[:, :],
                                     lhsT=s01[:, j * P:(j + 1) * P],
                                     rhs=r[:, j * 40:(j + 1) * 40],
                                     start=(j == 0), stop=(j == CPT - 1))
                    nc.tensor.matmul(out=den_ps[:, 0:1],
                                     lhsT=s01[:, j * P:(j + 1) * P],
                                     rhs=w[:, j:j + 1],
                                     start=(j == 0), stop=(j == CPT - 1))
                den = pd_ep.tile([P, 1], f32, tag='den2')
                nc.vector.tensor_scalar(out=den, in0=den_ps[:, 0:1],
                                        scalar1=1e-20, scalar2=None,
                                        op0=mybir.AluOpType.max)
                rec = pd_ep.tile([P, 1], f32, tag='rec2')
                nc.vector.reciprocal(out=rec, in_=den)
                o = pd_ep.tile([P, 40], f32, tag='o')
                nc.scalar.activation(o, acc[:, 0:40],
                                     mybir.ActivationFunctionType.Copy,
                                     scale=rec)
                ob = pd_ep.tile([P, 40], f32, tag='ob')
                nc.vector.tensor_tensor(out=ob, in0=o, in1=s_b2,
                                        op=mybir.AluOpType.add)
                ex = pd_ep.tile([P, 40], f32, tag='ex')
                sm = pd_ep.tile([P, 1], f32, tag='sm')
                nc.scalar.activation(ex, ob, mybir.ActivationFunctionType.Exp,
                                     accum_out=sm)
                rsm = pd_ep.tile([P, 1], f32, tag='rsm')
                nc.vector.reciprocal(out=rsm, in_=sm)
                nlg = pd_ep.tile([P, 1], f32, tag='nlg')
                nc.scalar.activation(nlg, rsm,
                                     mybir.ActivationFunctionType.Ln)
                fin = pd_ep.tile([P, 40], f32, tag='fin')
                nc.scalar.activation(fin, ob,
                                     mybir.ActivationFunctionType.Identity,
                                     bias=nlg)
                nc.sync.dma_start(out=out_d[t * P:(t + 1) * P, :], in_=fin)

    nc.compile()
    return nc


# ----------------------------------------------------------------- entry

_CACHE = {}


def prepare(x, edge_index, W1, att_src1, att_dst1, b1, W2, att_src2, att_dst2,
            b2, build=True, debug=False):
    x = np.asarray(x, F32)
    edge_index = np.asarray(edge_index)
    n_nodes = x.shape[0]

    meta, shapes, nstart, ncnt = _prep(edge_index, n_nodes)
    tables = _host_tables(x, np.asarray(W1, F32), np.asarray(att_src1, F32),
                          np.asarray(att_dst1, F32), np.asarray(W2, F32),
                          np.asarray(att_src2, F32), np.asarray(att_dst2, F32),
                          shapes)
    nc = None
    if build:
        key = (shapes['NT'], shapes['THR'], shapes['RB'], n_nodes, debug)
        if key not in _CACHE:
            _CACHE[key] = _build(shapes, n_nodes, debug=debug)
        nc = _CACHE[key]

    b1bc = np.broadcast_to(np.asarray(b1, F32), (P, H * C)).copy()
    b2bc = np.broadcast_to(np.asarray(b2, F32), (P, 40)).copy()

    in_maps = []
    for c in range(NC):
        in_maps.append(dict(
            t1x=tables['t1x'], xT=tables['xT'], wsd=tables['wsd'],
            w1=tables['w1'], w2e=tables['w2e'], iota_k=tables['iota_k'],
            ident=tables['ident'], b1bc=b1bc, b2bc=b2bc,
            t2bias=meta['t2bias'][c],
            idxA1=meta['idxA1'][c], idxB1=meta['idxB1'][c],
            idxA2=meta['idxA2'][c], idxB2=meta['idxB2'][c],
            dstloc=meta['dstloc'][c], adoffs=meta['adoffs'][c],
            s01T=meta['s01T'][c], s01h=meta['s01h'][c],
        ))
    return dict(nc=nc, in_maps=in_maps, shapes=shapes, nstart=nstart,
                ncnt=ncnt, n_nodes=n_nodes)


def assemble(ctx_run, outs):
    NT = ctx_run['shapes']['NT']
    nstart, ncnt = ctx_run['nstart'], ctx_run['ncnt']
    out = np.zeros((ctx_run['n_nodes'], 40), F32)
    for c in range(NC):
        oc = outs[c]['out']
        for t in range(NT):
            cnt = int(ncnt[c, t])
            if cnt == 0:
                continue
            n0 = int(nstart[c, t])
            out[n0:n0 + cnt] = oc[t * P:t * P + cnt]
    return out


def kernel(x, edge_index, W1, att_src1, att_dst1, b1, W2, att_src2, att_dst2, b2):
    ctx_run = prepare(x, edge_index, W1, att_src1, att_dst1, b1,
                      W2, att_src2, att_dst2, b2)
    res = run_bass_kernel_spmd(ctx_run['nc'], ctx_run['in_maps'],
                               list(range(NC)))
    return assemble(ctx_run, res.results)


# revision 37
# speedup vs baseline: 2.0102x; 1.1974x over previous
"""Trainium2 Bass kernel for 2-layer GAT (nn_GAT_90460601188538).

Strategy: edges sorted by destination; destination nodes greedily packed
into 128-slot tiles; tiles split contiguously across 8 cores. Per
edge-chunk of 128, a one-hot selection matrix (iota == dst_slot) turns the
segmented softmax-sum and scatter-add into PE matmuls accumulating in
PSUM. Softmax runs without max subtraction (scores are O(1)) as
unnormalized sums plus one divide per node. Edge-source features are
fetched with dma_gather (int16 indices), so every table is split at one
global node threshold THR into two halves gathered separately; each tile's
chunks are partitioned into group A (src < THR) and group B. Pad edge
slots point at row 0 with dst slot 200, whose one-hot row is all zero, so
they contribute nothing. Layer-2 features are exchanged with an AllGather.
"""
import sys
sys.path.insert(0, '/opt/trn_rl_repo')
from contextlib import ExitStack

import numpy as np
import ml_dtypes

import concourse.bacc as bacc
import concourse.tile as tile
from concourse import bass, mybir, library_config
from concourse.bass_utils import run_bass_kernel_spmd

BF16 = ml_dtypes.bfloat16
F32 = np.float32

P = 128
NC = 8
CA = 12               # group-A chunks per tile (src < THR)
CB = 8                # group-B chunks per tile (src >= THR)
CPT = CA + CB
NEG_SLOPE = 0.2
NEG_BIG = -10000.0
ROW = 128             # table row elems (bf16) = 256B
H, C, F = 4, 64, 128
THR_CAP = 30720       # node split threshold cap (int16 table indexing)
B_STOP = 6            # debug: truncate phase-B body (1..6)


# ----------------------------------------------------------------- host prep

def _wrap_idx(flat):
    """[n] int -> dma_gather layout [128, n//16] int16 (16-wrap, replicated)."""
    n = flat.shape[0]
    out = np.zeros((P, n // 16), np.int16)
    cols = flat.reshape(n // 16, 16).T.astype(np.int16)   # [16, n//16]
    for rep in range(8):
        out[rep * 16:(rep + 1) * 16, :] = cols
    return out


def _prep(edge_index, n_nodes):
    src = np.concatenate([edge_index[0], np.arange(n_nodes, dtype=np.int64)])
    dst = np.concatenate([edge_index[1], np.arange(n_nodes, dtype=np.int64)])
    perm = np.argsort(dst, kind='stable')
    src_s = src[perm].astype(np.int64)
    dst_s = dst[perm].astype(np.int64)
    deg = np.bincount(dst_s, minlength=n_nodes)

    THR = min(THR_CAP, n_nodes)   # node split threshold (A: src < THR)
    capA, capB = CA * P, CB * P

    # greedy pack consecutive nodes: <=P nodes, <=capA A-edges, <=capB B-edges
    isB = (src_s >= THR)
    degA = np.bincount(dst_s[~isB], minlength=n_nodes)
    degB = deg - degA
    cumA = np.concatenate([[0], np.cumsum(degA)])
    cumB = np.concatenate([[0], np.cumsum(degB)])
    tiles = []
    n = 0
    while n < n_nodes:
        hiA = int(np.searchsorted(cumA, cumA[n] + capA, side='right')) - 1 - n
        hiB = int(np.searchsorted(cumB, cumB[n] + capB, side='right')) - 1 - n
        cnt = max(1, min(hiA, hiB, P, n_nodes - n))
        assert cumA[n + cnt] - cumA[n] <= capA
        assert cumB[n + cnt] - cumB[n] <= capB
        tiles.append((n, cnt))
        n += cnt
    T = len(tiles)
    NT = (T + 1 + NC - 1) // NC      # >=1 pad tile overall
    NCH = NT * CPT

    nrow1 = ((n_nodes + P - 1) // P) * P
    nrow2 = NC * NT * P

    # node -> (global slot row, tile)
    node_row = np.zeros(n_nodes, np.int64)
    nstart = np.zeros((NC, NT), np.int64)
    ncnt = np.zeros((NC, NT), np.int64)
    for gi, (n0, cnt) in enumerate(tiles):
        c, t = gi // NT, gi % NT
        nstart[c, t] = n0
        ncnt[c, t] = cnt
        node_row[n0:n0 + cnt] = gi * P + np.arange(cnt)
    RB = int(node_row[THR]) if THR < n_nodes else max(0, nrow2 - P)
    assert RB < 32768 and nrow2 - RB <= 32768, (RB, nrow2)
    assert nrow1 - THR <= 32768, (THR, nrow1)

    # per-core streams
    idxA1 = np.zeros((NC, P, NT * CA * 8), np.int16)
    idxB1 = np.zeros((NC, P, NT * CB * 8), np.int16)
    idxA2 = np.zeros((NC, P, NT * CA * 8), np.int16)
    idxB2 = np.zeros((NC, P, NT * CB * 8), np.int16)
    dstloc = np.full((NC, P, NCH), BF16(200.0), BF16)
    adoffs = np.zeros((NC, P, NT), np.int32)      # slot node ids (a_d tile)
    cnts = np.zeros((NC, P, NT * 2), np.int32)    # valid idx counts (A,B)
    t2bias = np.full((NC, P, NT * 2), NEG_BIG, F32)

    # edge ranges per tile
    epos = 0
    edge_of_tile = []
    for (n0, cnt) in tiles:
        e0 = epos
        ecnt = int(deg[n0:n0 + cnt].sum())
        edge_of_tile.append((e0, ecnt))
        epos += ecnt

    for gi, (n0, cnt) in enumerate(tiles):
        c, t = gi // NT, gi % NT
        e0, ecnt = edge_of_tile[gi]
        es = src_s[e0:e0 + ecnt]
        ed = dst_s[e0:e0 + ecnt]
        sl = (ed - n0).astype(np.int64)
        selB = es >= THR
        esA, slA = es[~selB], sl[~selB]
        esB, slB = es[selB], sl[selB]
        fa = np.zeros(capA, np.int64)
        fa[:len(esA)] = esA
        fb = np.zeros(capB, np.int64)
        fb[:len(esB)] = esB - THR
        cnts[c, :, 2 * t] = len(esA)
        cnts[c, :, 2 * t + 1] = len(esB)
        idxA1[c, :, t * CA * 8:(t + 1) * CA * 8] = _wrap_idx(fa)
        idxB1[c, :, t * CB * 8:(t + 1) * CB * 8] = _wrap_idx(fb)
        fa2 = np.zeros(capA, np.int64)
        fa2[:len(esA)] = node_row[esA]
        fb2 = np.zeros(capB, np.int64)
        fb2[:len(esB)] = node_row[esB] - RB
        idxA2[c, :, t * CA * 8:(t + 1) * CA * 8] = _wrap_idx(fa2)
        idxB2[c, :, t * CB * 8:(t + 1) * CB * 8] = _wrap_idx(fb2)
        # dstloc: chunks 0..CA-1 = A slots, CA..CPT-1 = B slots
        dl = np.full((CPT, P), 200.0, np.float64)
        ia = np.arange(len(esA))
        dl[ia // P, ia % P] = slA
        ib = np.arange(len(esB))
        dl[CA + ib // P, ib % P] = slB
        dstloc[c, :, t * CPT:(t + 1) * CPT] = dl.T.astype(BF16)
        ad = np.full(P, n_nodes, np.int64)    # pad slots -> dummy row
        ad[:cnt] = n0 + np.arange(cnt)
        adoffs[c, :, t] = ad
        t2bias[c, :, 2 * t:2 * t + 2] = 0.0

    # host-built transposed one-hot: s01T[d, (t,j,e)] = 1 iff dst slot of
    # edge slot (t,j,e) == d; lhsT for broadcasting a_d to edge slots.
    dvals = np.arange(P, dtype=np.float64)
    s01T = np.zeros((NC, P, NT * CPT * P), BF16)
    s01h = np.zeros((NC, P, NT * CPT * P), BF16)
    for c in range(NC):
        # dstloc[c] is [P(e), NCH(t,j)] -> oh [d, t*CPT+j, e]
        oh = (dstloc[c].astype(np.float64).T[None, :, :] == dvals[:, None, None])
        s01T[c] = np.ascontiguousarray(oh).reshape(P, -1).astype(BF16)
        # s01h[e, (t,j,d)] = oh[d, tj, e]
        s01h[c] = np.ascontiguousarray(oh.transpose(2, 1, 0)).reshape(P, -1).astype(BF16)

    meta = dict(idxA1=idxA1, idxB1=idxB1, idxA2=idxA2, idxB2=idxB2,
                dstloc=dstloc, adoffs=adoffs, t2bias=t2bias, s01T=s01T,
                s01h=s01h, cnts=cnts)
    shapes = dict(T=T, NT=NT, NCH=NCH, nrow1=nrow1, nrow2=nrow2,
                  THR=THR, RB=RB)
    return meta, shapes, nstart, ncnt


def _host_tables(x, W1, att_src1, att_dst1, W2, att_src2, att_dst2, shapes):
    n_nodes = x.shape[0]
    nrow1 = shapes['nrow1']

    t1x = np.zeros((nrow1, 2 * ROW), BF16)
    t1x[:n_nodes, :F] = x.astype(BF16)

    # host-computed attention logits (f32, exact)
    W1r = W1.reshape(F, H, C)
    ws_cols = np.einsum('fhc,hc->fh', W1r, att_src1)      # [F, H]
    wd_cols = np.einsum('fhc,hc->fh', W1r, att_dst1)      # [F, H]
    a_s = x @ ws_cols                                      # [N, H]
    a_d = x @ wd_cols                                      # [N, H]
    t1x[:n_nodes, F:F + 4] = a_s.astype(BF16)
    adtab = np.zeros((nrow1, 4), BF16)
    adtab[:n_nodes] = a_d.astype(BF16)

    w1 = W1.astype(BF16)                                   # [128, 256]
    ws2 = (W2 @ att_src2[0])[:, None]
    wd2 = (W2 @ att_dst2[0])[:, None]
    w2e = np.concatenate([W2, ws2, wd2], axis=1).astype(BF16)  # [256, 42]
    w2e_packed = np.concatenate([w2e[0:P], w2e[P:2 * P]], axis=1)  # [128, 84]

    ident = np.eye(P, dtype=BF16)
    return dict(t1x=t1x, adtab=adtab, w1=w1, w2e=w2e_packed, ident=ident)


# ------------------------------------------------------------- device program

def _build(shapes, n_nodes, debug=False, phases='ABCD'):
    B_STOP = globals()['B_STOP']
    NT, NCH = shapes['NT'], shapes['NCH']
    nrow1, nrow2 = shapes['nrow1'], shapes['nrow2']
    THR, RB = shapes['THR'], shapes['RB']
    NAT = nrow1
    bf = mybir.dt.bfloat16
    f32 = mybir.dt.float32
    i32 = mybir.dt.int32
    i16 = mybir.dt.int16
    AG_CHUNKS = 4
    assert NT % AG_CHUNKS == 0 or True

    nc = bacc.Bacc('TRN2', target_bir_lowering=False, debug=False,
                   num_devices=NC)

    def inp(name, shape, dt):
        return nc.dram_tensor(name, list(shape), dt, kind='ExternalInput').ap()

    t1x = inp('t1x', (nrow1, 2 * ROW), bf)
    adtab = inp('adtab', (nrow1, 4), bf).ap() if False else inp('adtab', (nrow1, 4), bf)
    w1 = inp('w1', (P, H * C), bf)
    w2e = inp('w2e', (P, 2 * 42), bf)
    ident = inp('ident', (P, P), bf)
    b1bc = inp('b1bc', (P, H * C), f32)
    b2bc = inp('b2bc', (P, 40), f32)
    t2bias = inp('t2bias', (P, NT * 2), f32)
    m_idxA1 = inp('idxA1', (P, NT * CA * 8), i16)
    m_idxB1 = inp('idxB1', (P, NT * CB * 8), i16)
    m_idxA2 = inp('idxA2', (P, NT * CA * 8), i16)
    m_idxB2 = inp('idxB2', (P, NT * CB * 8), i16)
    m_adoffs = inp('adoffs', (P, NT), i32)
    m_cnts = inp('cnts', (P, NT * 2), i32)
    m_s01T = inp('s01T', (P, NT * CPT * P), bf)
    m_s01h = inp('s01h', (P, NT * CPT * P), bf)

    out_d = nc.dram_tensor('out', [NT * P, 40], f32, kind='ExternalOutput').ap()

    dbg = {}

    with tile.TileContext(nc) as tc, ExitStack() as ctx:
        nc.gpsimd.load_library(library_config.mlp)
        dram = ctx.enter_context(tc.tile_pool(name='dram', bufs=1, space='DRAM'))
        t2_local = dram.tile([NT * P, ROW], bf)
        t2_full = dram.tile([nrow2, ROW], bf, addr_space='Shared')

        consts = ctx.enter_context(tc.tile_pool(name='consts', bufs=1))
        meta = ctx.enter_context(tc.tile_pool(name='meta', bufs=1))

        s_w1 = consts.tile([P, H * C], bf)
        nc.sync.dma_start(out=s_w1, in_=w1)
        s_w2e = consts.tile([P, 2 * 42], bf)
        nc.sync.dma_start(out=s_w2e, in_=w2e)
        s_ident = consts.tile([P, P], bf)
        nc.sync.dma_start(out=s_ident, in_=ident)
        s_b1 = consts.tile([P, H * C], f32)
        nc.sync.dma_start(out=s_b1, in_=b1bc)
        s_b2 = consts.tile([P, 40], f32)
        nc.sync.dma_start(out=s_b2, in_=b2bc)
        s_t2bias = consts.tile([P, NT * 2], f32)
        nc.sync.dma_start(out=s_t2bias, in_=t2bias)

        s_idxA1 = meta.tile([P, NT * CA * 8], i16)
        nc.sync.dma_start(out=s_idxA1, in_=m_idxA1)
        s_idxB1 = meta.tile([P, NT * CB * 8], i16)
        nc.sync.dma_start(out=s_idxB1, in_=m_idxB1)
        s_idxA2 = meta.tile([P, NT * CA * 8], i16)
        nc.sync.dma_start(out=s_idxA2, in_=m_idxA2)
        s_idxB2 = meta.tile([P, NT * CB * 8], i16)
        nc.sync.dma_start(out=s_idxB2, in_=m_idxB2)
        s_adoffs = meta.tile([P, NT], i32)
        nc.sync.dma_start(out=s_adoffs, in_=m_adoffs)
        s_cnts = meta.tile([P, NT * 2], i32)
        nc.sync.dma_start(out=s_cnts, in_=m_cnts)

        if 'Z' in phases:   # minimal: write zeros to out
            with tc.tile_pool(name='pz', bufs=1) as pz:
                zt = pz.tile([P, 40], f32)
                nc.vector.memset(zt, 0.0)
                for t in range(NT):
                    nc.sync.dma_start(out=out_d[t * P:(t + 1) * P, :], in_=zt)

        # ---------------- phase B: layer-1 edges + tile epilogue -----------
        with tc.tile_pool(name='pb', bufs=2) as pb, \
             tc.tile_pool(name='pb_rhs', bufs=2) as pb_rhs, \
             tc.tile_pool(name='pb_ep', bufs=2) as pb_ep, \
             tc.tile_pool(name='ps_acc', bufs=1, space='PSUM') as ps_acc, \
             tc.tile_pool(name='ps_sm', bufs=2, space='PSUM') as ps_sm, \
             tc.tile_pool(name='ps_ep', bufs=1, space='PSUM') as ps_ep:
            for _pf in range(2):
                gx0 = pb.tile([P, CPT, 2 * ROW], bf, tag='gx')
                nc.vector.memset(gx0, 0.0)
            for t in range(NT if 'B' in phases else 0):
                c0 = t * CPT
                gx = pb.tile([P, CPT, 2 * ROW], bf, tag='gx')
                nc.gpsimd.dma_gather(gx[:, 0:CA, :], t1x,
                                     s_idxA1[:, t * CA * 8:(t + 1) * CA * 8],
                                     CA * P, CA * P, 2 * ROW,
                                     single_packet=False)
                nc.gpsimd.dma_gather(gx[:, CA:CPT, :], t1x[THR:nrow1, :],
                                     s_idxB1[:, t * CB * 8:(t + 1) * CB * 8],
                                     CB * P, CB * P, 2 * ROW,
                                     single_packet=False)
                adt = pb.tile([P, 4], bf, tag='adt')
                nc.gpsimd.indirect_dma_start(
                    out=adt, out_offset=None, in_=adtab,
                    in_offset=bass.IndirectOffsetOnAxis(
                        ap=s_adoffs[:, t:t + 1], axis=0))
                if B_STOP < 2:
                    continue
                # one-hot S01 for the whole tile (host-built)
                s01 = pb.tile([P, CPT * P], bf, tag='s01')
                nc.sync.dma_start(
                    out=s01, in_=m_s01h[:, t * CPT * P:(t + 1) * CPT * P])
                # scores: a_d broadcast via host s01T matmuls into PSUM,
                # then one strided add of the gathered a_s strip
                s01Tt = pb_rhs.tile([P, CPT * P], bf, tag='s01Tt')
                nc.sync.dma_start(
                    out=s01Tt,
                    in_=m_s01T[:, t * CPT * P:(t + 1) * CPT * P])
                sstage = pb.tile([P, CPT * 4], bf, tag='sstage')
                pss = [ps_acc.tile([P, P], f32, tag=f'ph{h}',
                                   name=f'ph{h}')
                       for h in range(H)]
                den_ps = ps_acc.tile([P, 8], f32, tag='den')
                w = pb.tile([P, CPT * 4], bf, tag='w')
                if B_STOP < 3:
                    continue
                sc_ps = ps_sm.tile([P, CPT * 4], f32, tag='psT', name='scps')
                for j in range(CPT):
                    nc.tensor.matmul(out=sc_ps[:, j * 4:(j + 1) * 4],
                                     lhsT=s01Tt[:, j * P:(j + 1) * P],
                                     rhs=adt, start=True, stop=True)
                nc.vector.tensor_tensor(
                    out=sstage.rearrange('p (k e) -> p k e', e=4),
                    in0=gx[:, :, F:F + 4],
                    in1=sc_ps.rearrange('p (k e) -> p k e', e=4),
                    op=mybir.AluOpType.add)
                if B_STOP < 4:
                    continue
                st = pb.tile([P, CPT * 4], bf, tag='st')
                nc.vector.tensor_scalar(out=st, in0=sstage, scalar1=NEG_SLOPE,
                                        scalar2=None, op0=mybir.AluOpType.mult)
                sl = pb.tile([P, CPT * 4], bf, tag='sl')
                nc.vector.tensor_tensor(out=sl, in0=st, in1=sstage,
                                        op=mybir.AluOpType.max)
                nc.scalar.activation(w, sl, mybir.ActivationFunctionType.Exp)
                # rhs_h = w_h * x ; accumulate
                rhs = []
                w3 = w.rearrange('p (k e) -> p k e', e=4)
                for h in range(H):
                    r = pb_rhs.tile([P, CPT * P], bf, tag=f'rhs{h}')
                    nc.vector.tensor_tensor(
                        out=r.rearrange('p (k j) -> p k j', j=P),
                        in0=gx[:, :, 0:F],
                        in1=w3[:, :, h:h + 1].to_broadcast([P, CPT, P]),
                        op=mybir.AluOpType.mult)
                    rhs.append(r)
                if B_STOP < 5:
                    continue
                for j in range(CPT):
                    for h in range(H):
                        nc.tensor.matmul(
                            out=pss[h][:, 0:128],
                            lhsT=rhs[h][:, j * P:(j + 1) * P],
                            rhs=s01[:, j * P:(j + 1) * P],
                            start=(j == 0), stop=(j == CPT - 1))
                    nc.tensor.matmul(
                        out=den_ps[:, 0:4],
                        lhsT=s01[:, j * P:(j + 1) * P],
                        rhs=w[:, j * 4:(j + 1) * 4],
                        start=(j == 0), stop=(j == CPT - 1))
                # epilogue
                if B_STOP < 6:
                    continue
                den = pb_ep.tile([P, 4], f32, tag='den4')
                nc.vector.tensor_scalar(out=den, in0=den_ps[:, 0:4],
                                        scalar1=1e-20, scalar2=None,
                                        op0=mybir.AluOpType.max)
                rec = pb_ep.tile([P, 4], f32, tag='rec')
                nc.vector.reciprocal(out=rec, in_=den)
                out1 = ps_ep.tile([P, H * C], f32, tag='ep')
                for h in range(H):
                    asb = pb_ep.tile([P, P], bf, tag=f'an{h}')
                    nc.scalar.activation(asb, pss[h][:, 0:128],
                                         mybir.ActivationFunctionType.Copy)
                    nc.tensor.matmul(out=out1[:, h * C:(h + 1) * C],
                                     lhsT=asb,
                                     rhs=s_w1[:, h * C:(h + 1) * C],
                                     start=True, stop=True)
                zsc = pb_ep.tile([P, H * C], f32, tag='zsc')
                for h in range(H):
                    nc.scalar.activation(zsc[:, C * h:C * (h + 1)],
                                         out1[:, C * h:C * (h + 1)],
                                         mybir.ActivationFunctionType.Copy,
                                         scale=rec[:, h:h + 1])
                zb = pb_ep.tile([P, H * C], f32, tag='zb')
                nc.vector.tensor_tensor(out=zb, in0=zsc, in1=s_b1,
                                        op=mybir.AluOpType.add)
                zr = pb_ep.tile([P, H * C], f32, tag='zr')
                nc.scalar.activation(zr, zb, mybir.ActivationFunctionType.Relu)
                zm = pb_ep.tile([P, H * C], f32, tag='zm')
                nc.vector.tensor_tensor(out=zm, in0=zb, in1=zr,
                                        op=mybir.AluOpType.subtract)
                ze = pb_ep.tile([P, H * C], f32, tag='ze')
                nc.scalar.activation(ze, zm, mybir.ActivationFunctionType.Exp)
                hb = pb_ep.tile([P, H * C], bf, tag='hb')
                nc.vector.scalar_tensor_tensor(out=hb, in0=zr, scalar=-1.0,
                                               in1=ze,
                                               op0=mybir.AluOpType.add,
                                               op1=mybir.AluOpType.add)
                xw2 = ps_ep.tile([P, 42], f32, tag='ep', name='xw2')
                for kk in range(2):
                    psT3 = ps_sm.tile([P, P], bf, tag='psT', name='psT3')
                    nc.tensor.transpose(out=psT3, in_=hb[:, kk * P:(kk + 1) * P],
                                        identity=s_ident)
                    hT = pb_ep.tile([P, P], bf, tag=f'hT{kk}')
                    nc.vector.tensor_copy(out=hT, in_=psT3)
                    nc.tensor.matmul(out=xw2, lhsT=hT,
                                     rhs=s_w2e[:, kk * 42:(kk + 1) * 42],
                                     start=(kk == 0), stop=(kk == 1))
                t2r = pb_ep.tile([P, ROW], bf, tag='t2r')
                nc.vector.memset(t2r[:, 42:ROW], 0.0)
                nc.vector.tensor_copy(out=t2r[:, 0:40], in_=xw2[:, 0:40])
                nc.vector.tensor_tensor(out=t2r[:, 40:42], in0=xw2[:, 40:42],
                                        in1=s_t2bias[:, 2 * t:2 * t + 2],
                                        op=mybir.AluOpType.add)
                nc.sync.dma_start(out=t2_local[t * P:(t + 1) * P, :], in_=t2r)

        # ---------------- phase C: allgather (chunked) ----------------
        CHK = NT // AG_CHUNKS if NT % AG_CHUNKS == 0 else NT
        nch = NT // CHK
        for a in range(nch if 'C' in phases else 0):
            nc.gpsimd.collective_compute(
                'AllGather', mybir.AluOpType.bypass,
                ins=[t2_local[a * CHK * P:(a + 1) * CHK * P, :]],
                outs=[t2_full.rearrange('(c r) e -> c r e', c=NC)
                      [:, a * CHK * P:(a + 1) * CHK * P, :]],
                replica_groups=[list(range(NC))])

        # ---------------- phase D: layer-2 edges + log_softmax ------------
        with tc.tile_pool(name='pd', bufs=2) as pd, \
             tc.tile_pool(name='pd_rhs', bufs=2) as pd_rhs, \
             tc.tile_pool(name='pd_ep', bufs=2) as pd_ep, \
             tc.tile_pool(name='ps2', bufs=2, space='PSUM') as ps2, \
             tc.tile_pool(name='ps2_sm', bufs=2, space='PSUM') as ps2_sm:
            for _pf in range(2):
                g20 = pd.tile([P, CPT, ROW], bf, tag='g2')
                nc.vector.memset(g20, 0.0)
            for t in range(NT if 'D' in phases else 0):
                c0 = t * CPT
                g2 = pd.tile([P, CPT, ROW], bf, tag='g2')
                nc.gpsimd.dma_gather(g2[:, 0:CA, :], t2_full[:, :],
                                     s_idxA2[:, t * CA * 8:(t + 1) * CA * 8],
                                     CA * P, CA * P, ROW, single_packet=False)
                nc.gpsimd.dma_gather(g2[:, CA:CPT, :], t2_full[RB:nrow2, :],
                                     s_idxB2[:, t * CB * 8:(t + 1) * CB * 8],
                                     CB * P, CB * P, ROW, single_packet=False)
                ad2 = pd.tile([P, 2], bf, tag='ad2')
                nc.sync.dma_start(out=ad2,
                                  in_=t2_local[t * P:(t + 1) * P, 40:42])
                s01 = pd.tile([P, CPT * P], bf, tag='s01b')
                nc.sync.dma_start(
                    out=s01, in_=m_s01h[:, t * CPT * P:(t + 1) * CPT * P])
                s01Tt = pd_rhs.tile([P, CPT * P], bf, tag='s01Tt2')
                nc.sync.dma_start(
                    out=s01Tt,
                    in_=m_s01T[:, t * CPT * P:(t + 1) * CPT * P])
                sstage = pd.tile([P, CPT], bf, tag='sst2')
                acc = ps2.tile([P, 40], f32, tag='acc2')
                den_ps = ps2.tile([P, 8], f32, tag='den2p')
                sc2_ps = ps2_sm.tile([P, CPT], f32, tag='psT2', name='sc2ps')
                for j in range(CPT):
                    nc.tensor.matmul(out=sc2_ps[:, j:j + 1],
                                     lhsT=s01Tt[:, j * P:(j + 1) * P],
                                     rhs=ad2[:, 1:2], start=True, stop=True)
                nc.vector.tensor_tensor(
                    out=sstage.rearrange('p (k e) -> p k e', e=1),
                    in0=g2[:, :, 40:41],
                    in1=sc2_ps.rearrange('p (k e) -> p k e', e=1),
                    op=mybir.AluOpType.add)
                st = pd.tile([P, CPT], bf, tag='st2')
                nc.vector.tensor_scalar(out=st, in0=sstage, scalar1=NEG_SLOPE,
                                        scalar2=None, op0=mybir.AluOpType.mult)
                sl = pd.tile([P, CPT], bf, tag='sl2')
                nc.vector.tensor_tensor(out=sl, in0=st, in1=sstage,
                                        op=mybir.AluOpType.max)
                w = pd.tile([P, CPT], bf, tag='w2')
                nc.scalar.activation(w, sl, mybir.ActivationFunctionType.Exp)
                r = pd_rhs.tile([P, CPT * 40], bf, tag='rhs2')
                nc.vector.tensor_tensor(
                    out=r.rearrange('p (k j) -> p k j', j=40),
                    in0=g2[:, :, 0:40],
                    in1=w.rearrange('p (k o) -> p k o', o=1)
                    .to_broadcast([P, CPT, 40]),
                    op=mybir.AluOpType.mult)
                for j in range(CPT):
                    nc.tensor.matmul(out=acc[:, :],
                                     lhsT=s01[:, j * P:(j + 1) * P],
                                     rhs=r[:, j * 40:(j + 1) * 40],
                                     start=(j == 0), stop=(j == CPT - 1))
                    nc.tensor.matmul(out=den_ps[:, 0:1],
                                     lhsT=s01[:, j * P:(j + 1) * P],
                                     rhs=w[:, j:j + 1],
                                     start=(j == 0), stop=(j == CPT - 1))
                den = pd_ep.tile([P, 1], f32, tag='den2')
                nc.vector.tensor_scalar(out=den, in0=den_ps[:, 0:1],
                                        scalar1=1e-20, scalar2=None,
                                        op0=mybir.AluOpType.max)
                rec = pd_ep.tile([P, 1], f32, tag='rec2')
                nc.vector.reciprocal(out=rec, in_=den)
                o = pd_ep.tile([P, 40], f32, tag='o')
                nc.scalar.activation(o, acc[:, 0:40],
                                     mybir.ActivationFunctionType.Copy,
                                     scale=rec)
                ob = pd_ep.tile([P, 40], f32, tag='ob')
                nc.vector.tensor_tensor(out=ob, in0=o, in1=s_b2,
                                        op=mybir.AluOpType.add)
                ex = pd_ep.tile([P, 40], f32, tag='ex')
                sm = pd_ep.tile([P, 1], f32, tag='sm')
                nc.scalar.activation(ex, ob, mybir.ActivationFunctionType.Exp,
                                     accum_out=sm)
                rsm = pd_ep.tile([P, 1], f32, tag='rsm')
                nc.vector.reciprocal(out=rsm, in_=sm)
                nlg = pd_ep.tile([P, 1], f32, tag='nlg')
                nc.scalar.activation(nlg, rsm,
                                     mybir.ActivationFunctionType.Ln)
                fin = pd_ep.tile([P, 40], f32, tag='fin')
                nc.scalar.activation(fin, ob,
                                     mybir.ActivationFunctionType.Identity,
                                     bias=nlg)
                nc.sync.dma_start(out=out_d[t * P:(t + 1) * P, :], in_=fin)

    nc.compile()
    return nc


# ----------------------------------------------------------------- entry

_CACHE = {}


def prepare(x, edge_index, W1, att_src1, att_dst1, b1, W2, att_src2, att_dst2,
            b2, build=True, debug=False):
    x = np.asarray(x, F32)
    edge_index = np.asarray(edge_index)
    n_nodes = x.shape[0]

    meta, shapes, nstart, ncnt = _prep(edge_index, n_nodes)
    tables = _host_tables(x, np.asarray(W1, F32), np.asarray(att_src1, F32),
                          np.asarray(att_dst1, F32), np.asarray(W2, F32),
                          np.asarray(att_src2, F32), np.asarray(att_dst2, F32),
                          shapes)
    nc = None
    if build:
        key = (shapes['NT'], shapes['THR'], shapes['RB'], n_nodes, debug)
        if key not in _CACHE:
            _CACHE[key] = _build(shapes, n_nodes, debug=debug)
        nc = _CACHE[key]

    b1bc = np.broadcast_to(np.asarray(b1, F32), (P, H * C)).copy()
    b2bc = np.broadcast_to(np.asarray(b2, F32), (P, 40)).copy()

    in_maps = []
    for c in range(NC):
        in_maps.append(dict(
            t1x=tables['t1x'], adtab=tables['adtab'],
            w1=tables['w1'], w2e=tables['w2e'],
            ident=tables['ident'], b1bc=b1bc, b2bc=b2bc,
            t2bias=meta['t2bias'][c],
            idxA1=meta['idxA1'][c], idxB1=meta['idxB1'][c],
            idxA2=meta['idxA2'][c], idxB2=meta['idxB2'][c],
            adoffs=meta['adoffs'][c], cnts=meta['cnts'][c],
            s01T=meta['s01T'][c], s01h=meta['s01h'][c],
        ))
    return dict(nc=nc, in_maps=in_maps, shapes=shapes, nstart=nstart,
                ncnt=ncnt, n_nodes=n_nodes)


def assemble(ctx_run, outs):
    NT = ctx_run['shapes']['NT']
    nstart, ncnt = ctx_run['nstart'], ctx_run['ncnt']
    out = np.zeros((ctx_run['n_nodes'], 40), F32)
    for c in range(NC):
        oc = outs[c]['out']
        for t in range(NT):
            cnt = int(ncnt[c, t])
            if cnt == 0:
                continue
            n0 = int(nstart[c, t])
            out[n0:n0 + cnt] = oc[t * P:t * P + cnt]
    return out


def kernel(x, edge_index, W1, att_src1, att_dst1, b1, W2, att_src2, att_dst2, b2):
    ctx_run = prepare(x, edge_index, W1, att_src1, att_dst1, b1,
                      W2, att_src2, att_dst2, b2)
    res = run_bass_kernel_spmd(ctx_run['nc'], ctx_run['in_maps'],
                               list(range(NC)))
    return assemble(ctx_run, res.results)


# revision 38
# speedup vs baseline: 2.1262x; 1.0577x over previous
"""Trainium2 Bass kernel for 2-layer GAT (nn_GAT_90460601188538).

Strategy: edges sorted by destination; destination nodes greedily packed
into 128-slot tiles; tiles split contiguously across 8 cores. Per
edge-chunk of 128, a one-hot selection matrix (iota == dst_slot) turns the
segmented softmax-sum and scatter-add into PE matmuls accumulating in
PSUM. Softmax runs without max subtraction (scores are O(1)) as
unnormalized sums plus one divide per node. Edge-source features are
fetched with dma_gather (int16 indices), so every table is split at one
global node threshold THR into two halves gathered separately; each tile's
chunks are partitioned into group A (src < THR) and group B. Pad edge
slots point at row 0 with dst slot 200, whose one-hot row is all zero, so
they contribute nothing. Layer-2 features are exchanged with an AllGather.
"""
import sys
sys.path.insert(0, '/opt/trn_rl_repo')
from contextlib import ExitStack

import numpy as np
import ml_dtypes

import concourse.bacc as bacc
import concourse.tile as tile
from concourse import bass, mybir, library_config
from concourse.bass_utils import run_bass_kernel_spmd

BF16 = ml_dtypes.bfloat16
F32 = np.float32

P = 128
NC = 8
CA = 11               # group-A chunks per tile (src < THR)
CB = 7                # group-B chunks per tile (src >= THR)
CPT = CA + CB
NEG_SLOPE = 0.2
NEG_BIG = -10000.0
ROW = 128             # table row elems (bf16) = 256B
H, C, F = 4, 64, 128
THR_CAP = 30720       # node split threshold cap (int16 table indexing)
B_STOP = 6            # debug: truncate phase-B body (1..6)


# ----------------------------------------------------------------- host prep

def _wrap_idx(flat):
    """[n] int -> dma_gather layout [128, n//16] int16 (16-wrap, replicated)."""
    n = flat.shape[0]
    out = np.zeros((P, n // 16), np.int16)
    cols = flat.reshape(n // 16, 16).T.astype(np.int16)   # [16, n//16]
    for rep in range(8):
        out[rep * 16:(rep + 1) * 16, :] = cols
    return out


def _prep(edge_index, n_nodes):
    src = np.concatenate([edge_index[0], np.arange(n_nodes, dtype=np.int64)])
    dst = np.concatenate([edge_index[1], np.arange(n_nodes, dtype=np.int64)])
    perm = np.argsort(dst, kind='stable')
    src_s = src[perm].astype(np.int64)
    dst_s = dst[perm].astype(np.int64)
    deg = np.bincount(dst_s, minlength=n_nodes)

    THR = min(THR_CAP, n_nodes)   # node split threshold (A: src < THR)
    capA, capB = CA * P, CB * P

    # greedy pack consecutive nodes: <=P nodes, <=capA A-edges, <=capB B-edges
    isB = (src_s >= THR)
    degA = np.bincount(dst_s[~isB], minlength=n_nodes)
    degB = deg - degA
    cumA = np.concatenate([[0], np.cumsum(degA)])
    cumB = np.concatenate([[0], np.cumsum(degB)])
    tiles = []
    n = 0
    while n < n_nodes:
        hiA = int(np.searchsorted(cumA, cumA[n] + capA, side='right')) - 1 - n
        hiB = int(np.searchsorted(cumB, cumB[n] + capB, side='right')) - 1 - n
        cnt = max(1, min(hiA, hiB, P, n_nodes - n))
        assert cumA[n + cnt] - cumA[n] <= capA
        assert cumB[n + cnt] - cumB[n] <= capB
        tiles.append((n, cnt))
        n += cnt
    T = len(tiles)
    NT = (T + 1 + NC - 1) // NC      # >=1 pad tile overall
    NCH = NT * CPT

    nrow1 = ((n_nodes + P - 1) // P) * P
    nrow2 = NC * NT * P

    # node -> (global slot row, tile)
    node_row = np.zeros(n_nodes, np.int64)
    nstart = np.zeros((NC, NT), np.int64)
    ncnt = np.zeros((NC, NT), np.int64)
    for gi, (n0, cnt) in enumerate(tiles):
        c, t = gi // NT, gi % NT
        nstart[c, t] = n0
        ncnt[c, t] = cnt
        node_row[n0:n0 + cnt] = gi * P + np.arange(cnt)
    RB = int(node_row[THR]) if THR < n_nodes else max(0, nrow2 - P)
    assert RB < 32768 and nrow2 - RB <= 32768, (RB, nrow2)
    assert nrow1 - THR <= 32768, (THR, nrow1)

    # per-core streams
    idxA1 = np.zeros((NC, P, NT * CA * 8), np.int16)
    idxB1 = np.zeros((NC, P, NT * CB * 8), np.int16)
    idxA2 = np.zeros((NC, P, NT * CA * 8), np.int16)
    idxB2 = np.zeros((NC, P, NT * CB * 8), np.int16)
    dstloc = np.full((NC, P, NCH), BF16(200.0), BF16)
    adoffs = np.zeros((NC, P, NT), np.int32)      # slot node ids (a_d tile)
    cnts = np.zeros((NC, P, NT * 2), np.int32)    # valid idx counts (A,B)
    t2bias = np.full((NC, P, NT * 2), NEG_BIG, F32)

    # edge ranges per tile
    epos = 0
    edge_of_tile = []
    for (n0, cnt) in tiles:
        e0 = epos
        ecnt = int(deg[n0:n0 + cnt].sum())
        edge_of_tile.append((e0, ecnt))
        epos += ecnt

    for gi, (n0, cnt) in enumerate(tiles):
        c, t = gi // NT, gi % NT
        e0, ecnt = edge_of_tile[gi]
        es = src_s[e0:e0 + ecnt]
        ed = dst_s[e0:e0 + ecnt]
        sl = (ed - n0).astype(np.int64)
        selB = es >= THR
        esA, slA = es[~selB], sl[~selB]
        esB, slB = es[selB], sl[selB]
        fa = np.zeros(capA, np.int64)
        fa[:len(esA)] = esA
        fb = np.zeros(capB, np.int64)
        fb[:len(esB)] = esB - THR
        cnts[c, :, 2 * t] = len(esA)
        cnts[c, :, 2 * t + 1] = len(esB)
        idxA1[c, :, t * CA * 8:(t + 1) * CA * 8] = _wrap_idx(fa)
        idxB1[c, :, t * CB * 8:(t + 1) * CB * 8] = _wrap_idx(fb)
        fa2 = np.zeros(capA, np.int64)
        fa2[:len(esA)] = node_row[esA]
        fb2 = np.zeros(capB, np.int64)
        fb2[:len(esB)] = node_row[esB] - RB
        idxA2[c, :, t * CA * 8:(t + 1) * CA * 8] = _wrap_idx(fa2)
        idxB2[c, :, t * CB * 8:(t + 1) * CB * 8] = _wrap_idx(fb2)
        # dstloc: chunks 0..CA-1 = A slots, CA..CPT-1 = B slots
        dl = np.full((CPT, P), 200.0, np.float64)
        ia = np.arange(len(esA))
        dl[ia // P, ia % P] = slA
        ib = np.arange(len(esB))
        dl[CA + ib // P, ib % P] = slB
        dstloc[c, :, t * CPT:(t + 1) * CPT] = dl.T.astype(BF16)
        ad = np.full(P, n_nodes, np.int64)    # pad slots -> dummy row
        ad[:cnt] = n0 + np.arange(cnt)
        adoffs[c, :, t] = ad
        t2bias[c, :, 2 * t:2 * t + 2] = 0.0

    # host-built transposed one-hot: s01T[d, (t,j,e)] = 1 iff dst slot of
    # edge slot (t,j,e) == d; lhsT for broadcasting a_d to edge slots.
    dvals = np.arange(P, dtype=np.float64)
    s01T = np.zeros((NC, P, NT * CPT * P), BF16)
    s01h = np.zeros((NC, P, NT * CPT * P), BF16)
    for c in range(NC):
        # dstloc[c] is [P(e), NCH(t,j)] -> oh [d, t*CPT+j, e]
        oh = (dstloc[c].astype(np.float64).T[None, :, :] == dvals[:, None, None])
        s01T[c] = np.ascontiguousarray(oh).reshape(P, -1).astype(BF16)
        # s01h[e, (t,j,d)] = oh[d, tj, e]
        s01h[c] = np.ascontiguousarray(oh.transpose(2, 1, 0)).reshape(P, -1).astype(BF16)

    meta = dict(idxA1=idxA1, idxB1=idxB1, idxA2=idxA2, idxB2=idxB2,
                dstloc=dstloc, adoffs=adoffs, t2bias=t2bias, s01T=s01T,
                s01h=s01h, cnts=cnts)
    shapes = dict(T=T, NT=NT, NCH=NCH, nrow1=nrow1, nrow2=nrow2,
                  THR=THR, RB=RB)
    return meta, shapes, nstart, ncnt


def _host_tables(x, W1, att_src1, att_dst1, W2, att_src2, att_dst2, shapes):
    n_nodes = x.shape[0]
    nrow1 = shapes['nrow1']

    t1x = np.zeros((nrow1, 2 * ROW), BF16)
    t1x[:n_nodes, :F] = x.astype(BF16)

    # host-computed attention logits (f32, exact)
    W1r = W1.reshape(F, H, C)
    ws_cols = np.einsum('fhc,hc->fh', W1r, att_src1)      # [F, H]
    wd_cols = np.einsum('fhc,hc->fh', W1r, att_dst1)      # [F, H]
    a_s = x @ ws_cols                                      # [N, H]
    a_d = x @ wd_cols                                      # [N, H]
    t1x[:n_nodes, F:F + 4] = a_s.astype(BF16)
    adtab = np.zeros((nrow1, 4), BF16)
    adtab[:n_nodes] = a_d.astype(BF16)

    w1 = W1.astype(BF16)                                   # [128, 256]
    ws2 = (W2 @ att_src2[0])[:, None]
    wd2 = (W2 @ att_dst2[0])[:, None]
    w2e = np.concatenate([W2, ws2, wd2], axis=1).astype(BF16)  # [256, 42]
    w2e_packed = np.concatenate([w2e[0:P], w2e[P:2 * P]], axis=1)  # [128, 84]

    ident = np.eye(P, dtype=BF16)
    return dict(t1x=t1x, adtab=adtab, w1=w1, w2e=w2e_packed, ident=ident)


# ------------------------------------------------------------- device program

def _build(shapes, n_nodes, debug=False, phases='ABCD'):
    B_STOP = globals()['B_STOP']
    NT, NCH = shapes['NT'], shapes['NCH']
    nrow1, nrow2 = shapes['nrow1'], shapes['nrow2']
    THR, RB = shapes['THR'], shapes['RB']
    NAT = nrow1
    bf = mybir.dt.bfloat16
    f32 = mybir.dt.float32
    i32 = mybir.dt.int32
    i16 = mybir.dt.int16
    AG_CHUNKS = 4
    assert NT % AG_CHUNKS == 0 or True

    nc = bacc.Bacc('TRN2', target_bir_lowering=False, debug=False,
                   num_devices=NC)

    def inp(name, shape, dt):
        return nc.dram_tensor(name, list(shape), dt, kind='ExternalInput').ap()

    t1x = inp('t1x', (nrow1, 2 * ROW), bf)
    adtab = inp('adtab', (nrow1, 4), bf).ap() if False else inp('adtab', (nrow1, 4), bf)
    w1 = inp('w1', (P, H * C), bf)
    w2e = inp('w2e', (P, 2 * 42), bf)
    ident = inp('ident', (P, P), bf)
    b1bc = inp('b1bc', (P, H * C), f32)
    b2bc = inp('b2bc', (P, 40), f32)
    t2bias = inp('t2bias', (P, NT * 2), f32)
    m_idxA1 = inp('idxA1', (P, NT * CA * 8), i16)
    m_idxB1 = inp('idxB1', (P, NT * CB * 8), i16)
    m_idxA2 = inp('idxA2', (P, NT * CA * 8), i16)
    m_idxB2 = inp('idxB2', (P, NT * CB * 8), i16)
    m_adoffs = inp('adoffs', (P, NT), i32)
    m_cnts = inp('cnts', (P, NT * 2), i32)
    m_s01T = inp('s01T', (P, NT * CPT * P), bf)
    m_s01h = inp('s01h', (P, NT * CPT * P), bf)

    out_d = nc.dram_tensor('out', [NT * P, 40], f32, kind='ExternalOutput').ap()

    dbg = {}

    with tile.TileContext(nc) as tc, ExitStack() as ctx:
        nc.gpsimd.load_library(library_config.mlp)
        dram = ctx.enter_context(tc.tile_pool(name='dram', bufs=1, space='DRAM'))
        t2_local = dram.tile([NT * P, ROW], bf)
        t2_full = dram.tile([nrow2, ROW], bf, addr_space='Shared')

        consts = ctx.enter_context(tc.tile_pool(name='consts', bufs=1))
        meta = ctx.enter_context(tc.tile_pool(name='meta', bufs=1))

        s_w1 = consts.tile([P, H * C], bf)
        nc.sync.dma_start(out=s_w1, in_=w1)
        s_w2e = consts.tile([P, 2 * 42], bf)
        nc.sync.dma_start(out=s_w2e, in_=w2e)
        s_ident = consts.tile([P, P], bf)
        nc.sync.dma_start(out=s_ident, in_=ident)
        s_b1 = consts.tile([P, H * C], f32)
        nc.sync.dma_start(out=s_b1, in_=b1bc)
        s_b2 = consts.tile([P, 40], f32)
        nc.sync.dma_start(out=s_b2, in_=b2bc)
        s_t2bias = consts.tile([P, NT * 2], f32)
        nc.sync.dma_start(out=s_t2bias, in_=t2bias)

        s_idxA1 = meta.tile([P, NT * CA * 8], i16)
        nc.sync.dma_start(out=s_idxA1, in_=m_idxA1)
        s_idxB1 = meta.tile([P, NT * CB * 8], i16)
        nc.sync.dma_start(out=s_idxB1, in_=m_idxB1)
        s_idxA2 = meta.tile([P, NT * CA * 8], i16)
        nc.sync.dma_start(out=s_idxA2, in_=m_idxA2)
        s_idxB2 = meta.tile([P, NT * CB * 8], i16)
        nc.sync.dma_start(out=s_idxB2, in_=m_idxB2)
        s_adoffs = meta.tile([P, NT], i32)
        nc.sync.dma_start(out=s_adoffs, in_=m_adoffs)
        s_cnts = meta.tile([P, NT * 2], i32)
        nc.sync.dma_start(out=s_cnts, in_=m_cnts)

        if 'Z' in phases:   # minimal: write zeros to out
            with tc.tile_pool(name='pz', bufs=1) as pz:
                zt = pz.tile([P, 40], f32)
                nc.vector.memset(zt, 0.0)
                for t in range(NT):
                    nc.sync.dma_start(out=out_d[t * P:(t + 1) * P, :], in_=zt)

        # ---------------- phase B: layer-1 edges + tile epilogue -----------
        with tc.tile_pool(name='pb', bufs=2) as pb, \
             tc.tile_pool(name='pb_rhs', bufs=2) as pb_rhs, \
             tc.tile_pool(name='pb_ep', bufs=2) as pb_ep, \
             tc.tile_pool(name='ps_acc', bufs=1, space='PSUM') as ps_acc, \
             tc.tile_pool(name='ps_sm', bufs=2, space='PSUM') as ps_sm, \
             tc.tile_pool(name='ps_ep', bufs=1, space='PSUM') as ps_ep:
            for _pf in range(2):
                gx0 = pb.tile([P, CPT, 2 * ROW], bf, tag='gx')
                nc.vector.memset(gx0, 0.0)
            for t in range(NT if 'B' in phases else 0):
                c0 = t * CPT
                gx = pb.tile([P, CPT, 2 * ROW], bf, tag='gx')
                nc.gpsimd.dma_gather(gx[:, 0:CA, :], t1x,
                                     s_idxA1[:, t * CA * 8:(t + 1) * CA * 8],
                                     CA * P, CA * P, 2 * ROW,
                                     single_packet=False)
                nc.gpsimd.dma_gather(gx[:, CA:CPT, :], t1x[THR:nrow1, :],
                                     s_idxB1[:, t * CB * 8:(t + 1) * CB * 8],
                                     CB * P, CB * P, 2 * ROW,
                                     single_packet=False)
                adt = pb.tile([P, 4], bf, tag='adt')
                nc.gpsimd.indirect_dma_start(
                    out=adt, out_offset=None, in_=adtab,
                    in_offset=bass.IndirectOffsetOnAxis(
                        ap=s_adoffs[:, t:t + 1], axis=0))
                if B_STOP < 2:
                    continue
                # one-hot S01 for the whole tile (host-built)
                s01 = pb.tile([P, CPT * P], bf, tag='s01')
                nc.sync.dma_start(
                    out=s01, in_=m_s01h[:, t * CPT * P:(t + 1) * CPT * P])
                # scores: a_d broadcast via host s01T matmuls into PSUM,
                # then one strided add of the gathered a_s strip
                s01Tt = pb_rhs.tile([P, CPT * P], bf, tag='s01Tt')
                nc.sync.dma_start(
                    out=s01Tt,
                    in_=m_s01T[:, t * CPT * P:(t + 1) * CPT * P])
                sstage = pb.tile([P, CPT * 4], bf, tag='sstage')
                pss = [ps_acc.tile([P, P], f32, tag=f'ph{h}',
                                   name=f'ph{h}')
                       for h in range(H)]
                den_ps = ps_acc.tile([P, 8], f32, tag='den')
                w = pb.tile([P, CPT * 4], bf, tag='w')
                if B_STOP < 3:
                    continue
                sc_ps = ps_sm.tile([P, CPT * 4], f32, tag='psT', name='scps')
                for j in range(CPT):
                    nc.tensor.matmul(out=sc_ps[:, j * 4:(j + 1) * 4],
                                     lhsT=s01Tt[:, j * P:(j + 1) * P],
                                     rhs=adt, start=True, stop=True)
                nc.vector.tensor_tensor(
                    out=sstage.rearrange('p (k e) -> p k e', e=4),
                    in0=gx[:, :, F:F + 4],
                    in1=sc_ps.rearrange('p (k e) -> p k e', e=4),
                    op=mybir.AluOpType.add)
                if B_STOP < 4:
                    continue
                st = pb.tile([P, CPT * 4], bf, tag='st')
                nc.vector.tensor_scalar(out=st, in0=sstage, scalar1=NEG_SLOPE,
                                        scalar2=None, op0=mybir.AluOpType.mult)
                sl = pb.tile([P, CPT * 4], bf, tag='sl')
                nc.vector.tensor_tensor(out=sl, in0=st, in1=sstage,
                                        op=mybir.AluOpType.max)
                nc.scalar.activation(w, sl, mybir.ActivationFunctionType.Exp)
                # rhs_h = w_h * x ; accumulate
                rhs = []
                w3 = w.rearrange('p (k e) -> p k e', e=4)
                for h in range(H):
                    r = pb_rhs.tile([P, CPT * P], bf, tag=f'rhs{h}')
                    nc.vector.tensor_tensor(
                        out=r.rearrange('p (k j) -> p k j', j=P),
                        in0=gx[:, :, 0:F],
                        in1=w3[:, :, h:h + 1].to_broadcast([P, CPT, P]),
                        op=mybir.AluOpType.mult)
                    rhs.append(r)
                if B_STOP < 5:
                    continue
                for j in range(CPT):
                    for h in range(H):
                        nc.tensor.matmul(
                            out=pss[h][:, 0:128],
                            lhsT=rhs[h][:, j * P:(j + 1) * P],
                            rhs=s01[:, j * P:(j + 1) * P],
                            start=(j == 0), stop=(j == CPT - 1))
                    nc.tensor.matmul(
                        out=den_ps[:, 0:4],
                        lhsT=s01[:, j * P:(j + 1) * P],
                        rhs=w[:, j * 4:(j + 1) * 4],
                        start=(j == 0), stop=(j == CPT - 1))
                # epilogue
                if B_STOP < 6:
                    continue
                den = pb_ep.tile([P, 4], f32, tag='den4')
                nc.vector.tensor_scalar(out=den, in0=den_ps[:, 0:4],
                                        scalar1=1e-20, scalar2=None,
                                        op0=mybir.AluOpType.max)
                rec = pb_ep.tile([P, 4], f32, tag='rec')
                nc.vector.reciprocal(out=rec, in_=den)
                out1 = ps_ep.tile([P, H * C], f32, tag='ep')
                for h in range(H):
                    asb = pb_ep.tile([P, P], bf, tag=f'an{h}')
                    nc.scalar.activation(asb, pss[h][:, 0:128],
                                         mybir.ActivationFunctionType.Copy)
                    nc.tensor.matmul(out=out1[:, h * C:(h + 1) * C],
                                     lhsT=asb,
                                     rhs=s_w1[:, h * C:(h + 1) * C],
                                     start=True, stop=True)
                zsc = pb_ep.tile([P, H * C], f32, tag='zsc')
                for h in range(H):
                    nc.scalar.activation(zsc[:, C * h:C * (h + 1)],
                                         out1[:, C * h:C * (h + 1)],
                                         mybir.ActivationFunctionType.Copy,
                                         scale=rec[:, h:h + 1])
                zb = pb_ep.tile([P, H * C], f32, tag='zb')
                nc.vector.tensor_tensor(out=zb, in0=zsc, in1=s_b1,
                                        op=mybir.AluOpType.add)
                zr = pb_ep.tile([P, H * C], f32, tag='zr')
                nc.scalar.activation(zr, zb, mybir.ActivationFunctionType.Relu)
                zm = pb_ep.tile([P, H * C], f32, tag='zm')
                nc.vector.tensor_tensor(out=zm, in0=zb, in1=zr,
                                        op=mybir.AluOpType.subtract)
                ze = pb_ep.tile([P, H * C], f32, tag='ze')
                nc.scalar.activation(ze, zm, mybir.ActivationFunctionType.Exp)
                hb = pb_ep.tile([P, H * C], bf, tag='hb')
                nc.vector.scalar_tensor_tensor(out=hb, in0=zr, scalar=-1.0,
                                               in1=ze,
                                               op0=mybir.AluOpType.add,
                                               op1=mybir.AluOpType.add)
                xw2 = ps_ep.tile([P, 42], f32, tag='ep', name='xw2')
                for kk in range(2):
                    psT3 = ps_sm.tile([P, P], bf, tag='psT', name='psT3')
                    nc.tensor.transpose(out=psT3, in_=hb[:, kk * P:(kk + 1) * P],
                                        identity=s_ident)
                    hT = pb_ep.tile([P, P], bf, tag=f'hT{kk}')
                    nc.vector.tensor_copy(out=hT, in_=psT3)
                    nc.tensor.matmul(out=xw2, lhsT=hT,
                                     rhs=s_w2e[:, kk * 42:(kk + 1) * 42],
                                     start=(kk == 0), stop=(kk == 1))
                t2r = pb_ep.tile([P, ROW], bf, tag='t2r')
                nc.vector.memset(t2r[:, 42:ROW], 0.0)
                nc.vector.tensor_copy(out=t2r[:, 0:40], in_=xw2[:, 0:40])
                nc.vector.tensor_tensor(out=t2r[:, 40:42], in0=xw2[:, 40:42],
                                        in1=s_t2bias[:, 2 * t:2 * t + 2],
                                        op=mybir.AluOpType.add)
                nc.sync.dma_start(out=t2_local[t * P:(t + 1) * P, :], in_=t2r)

        # ---------------- phase C: allgather (chunked) ----------------
        CHK = NT // AG_CHUNKS if NT % AG_CHUNKS == 0 else NT
        nch = NT // CHK
        for a in range(nch if 'C' in phases else 0):
            nc.gpsimd.collective_compute(
                'AllGather', mybir.AluOpType.bypass,
                ins=[t2_local[a * CHK * P:(a + 1) * CHK * P, :]],
                outs=[t2_full.rearrange('(c r) e -> c r e', c=NC)
                      [:, a * CHK * P:(a + 1) * CHK * P, :]],
                replica_groups=[list(range(NC))])

        # ---------------- phase D: layer-2 edges + log_softmax ------------
        with tc.tile_pool(name='pd', bufs=2) as pd, \
             tc.tile_pool(name='pd_rhs', bufs=2) as pd_rhs, \
             tc.tile_pool(name='pd_ep', bufs=2) as pd_ep, \
             tc.tile_pool(name='ps2', bufs=2, space='PSUM') as ps2, \
             tc.tile_pool(name='ps2_sm', bufs=2, space='PSUM') as ps2_sm:
            for _pf in range(2):
                g20 = pd.tile([P, CPT, ROW], bf, tag='g2')
                nc.vector.memset(g20, 0.0)
            for t in range(NT if 'D' in phases else 0):
                c0 = t * CPT
                g2 = pd.tile([P, CPT, ROW], bf, tag='g2')
                nc.gpsimd.dma_gather(g2[:, 0:CA, :], t2_full[:, :],
                                     s_idxA2[:, t * CA * 8:(t + 1) * CA * 8],
                                     CA * P, CA * P, ROW, single_packet=False)
                nc.gpsimd.dma_gather(g2[:, CA:CPT, :], t2_full[RB:nrow2, :],
                                     s_idxB2[:, t * CB * 8:(t + 1) * CB * 8],
                                     CB * P, CB * P, ROW, single_packet=False)
                ad2 = pd.tile([P, 2], bf, tag='ad2')
                nc.sync.dma_start(out=ad2,
                                  in_=t2_local[t * P:(t + 1) * P, 40:42])
                s01 = pd.tile([P, CPT * P], bf, tag='s01b')
                nc.sync.dma_start(
                    out=s01, in_=m_s01h[:, t * CPT * P:(t + 1) * CPT * P])
                s01Tt = pd_rhs.tile([P, CPT * P], bf, tag='s01Tt2')
                nc.sync.dma_start(
                    out=s01Tt,
                    in_=m_s01T[:, t * CPT * P:(t + 1) * CPT * P])
                sstage = pd.tile([P, CPT], bf, tag='sst2')
                acc = ps2.tile([P, 40], f32, tag='acc2')
                den_ps = ps2.tile([P, 8], f32, tag='den2p')
                sc2_ps = ps2_sm.tile([P, CPT], f32, tag='psT2', name='sc2ps')
                for j in range(CPT):
                    nc.tensor.matmul(out=sc2_ps[:, j:j + 1],
                                     lhsT=s01Tt[:, j * P:(j + 1) * P],
                                     rhs=ad2[:, 1:2], start=True, stop=True)
                nc.vector.tensor_tensor(
                    out=sstage.rearrange('p (k e) -> p k e', e=1),
                    in0=g2[:, :, 40:41],
                    in1=sc2_ps.rearrange('p (k e) -> p k e', e=1),
                    op=mybir.AluOpType.add)
                st = pd.tile([P, CPT], bf, tag='st2')
                nc.vector.tensor_scalar(out=st, in0=sstage, scalar1=NEG_SLOPE,
                                        scalar2=None, op0=mybir.AluOpType.mult)
                sl = pd.tile([P, CPT], bf, tag='sl2')
                nc.vector.tensor_tensor(out=sl, in0=st, in1=sstage,
                                        op=mybir.AluOpType.max)
                w = pd.tile([P, CPT], bf, tag='w2')
                nc.scalar.activation(w, sl, mybir.ActivationFunctionType.Exp)
                r = pd_rhs.tile([P, CPT * 40], bf, tag='rhs2')
                nc.vector.tensor_tensor(
                    out=r.rearrange('p (k j) -> p k j', j=40),
                    in0=g2[:, :, 0:40],
                    in1=w.rearrange('p (k o) -> p k o', o=1)
                    .to_broadcast([P, CPT, 40]),
                    op=mybir.AluOpType.mult)
                for j in range(CPT):
                    nc.tensor.matmul(out=acc[:, :],
                                     lhsT=s01[:, j * P:(j + 1) * P],
                                     rhs=r[:, j * 40:(j + 1) * 40],
                                     start=(j == 0), stop=(j == CPT - 1))
                    nc.tensor.matmul(out=den_ps[:, 0:1],
                                     lhsT=s01[:, j * P:(j + 1) * P],
                                     rhs=w[:, j:j + 1],
                                     start=(j == 0), stop=(j == CPT - 1))
                den = pd_ep.tile([P, 1], f32, tag='den2')
                nc.vector.tensor_scalar(out=den, in0=den_ps[:, 0:1],
                                        scalar1=1e-20, scalar2=None,
                                        op0=mybir.AluOpType.max)
                rec = pd_ep.tile([P, 1], f32, tag='rec2')
                nc.vector.reciprocal(out=rec, in_=den)
                o = pd_ep.tile([P, 40], f32, tag='o')
                nc.scalar.activation(o, acc[:, 0:40],
                                     mybir.ActivationFunctionType.Copy,
                                     scale=rec)
                ob = pd_ep.tile([P, 40], f32, tag='ob')
                nc.vector.tensor_tensor(out=ob, in0=o, in1=s_b2,
                                        op=mybir.AluOpType.add)
                ex = pd_ep.tile([P, 40], f32, tag='ex')
                sm = pd_ep.tile([P, 1], f32, tag='sm')
                nc.scalar.activation(ex, ob, mybir.ActivationFunctionType.Exp,
                                     accum_out=sm)
                rsm = pd_ep.tile([P, 1], f32, tag='rsm')
                nc.vector.reciprocal(out=rsm, in_=sm)
                nlg = pd_ep.tile([P, 1], f32, tag='nlg')
                nc.scalar.activation(nlg, rsm,
                                     mybir.ActivationFunctionType.Ln)
                fin = pd_ep.tile([P, 40], f32, tag='fin')
                nc.scalar.activation(fin, ob,
                                     mybir.ActivationFunctionType.Identity,
                                     bias=nlg)
                nc.sync.dma_start(out=out_d[t * P:(t + 1) * P, :], in_=fin)

    nc.compile()
    return nc


# ----------------------------------------------------------------- entry

_CACHE = {}


def prepare(x, edge_index, W1, att_src1, att_dst1, b1, W2, att_src2, att_dst2,
            b2, build=True, debug=False):
    x = np.asarray(x, F32)
    edge_index = np.asarray(edge_index)
    n_nodes = x.shape[0]

    meta, shapes, nstart, ncnt = _prep(edge_index, n_nodes)
    tables = _host_tables(x, np.asarray(W1, F32), np.asarray(att_src1, F32),
                          np.asarray(att_dst1, F32), np.asarray(W2, F32),
                          np.asarray(att_src2, F32), np.asarray(att_dst2, F32),
                          shapes)
    nc = None
    if build:
        key = (shapes['NT'], shapes['THR'], shapes['RB'], n_nodes, debug)
        if key not in _CACHE:
            _CACHE[key] = _build(shapes, n_nodes, debug=debug)
        nc = _CACHE[key]

    b1bc = np.broadcast_to(np.asarray(b1, F32), (P, H * C)).copy()
    b2bc = np.broadcast_to(np.asarray(b2, F32), (P, 40)).copy()

    in_maps = []
    for c in range(NC):
        in_maps.append(dict(
            t1x=tables['t1x'], adtab=tables['adtab'],
            w1=tables['w1'], w2e=tables['w2e'],
            ident=tables['ident'], b1bc=b1bc, b2bc=b2bc,
            t2bias=meta['t2bias'][c],
            idxA1=meta['idxA1'][c], idxB1=meta['idxB1'][c],
            idxA2=meta['idxA2'][c], idxB2=meta['idxB2'][c],
            adoffs=meta['adoffs'][c], cnts=meta['cnts'][c],
            s01T=meta['s01T'][c], s01h=meta['s01h'][c],
        ))
    return dict(nc=nc, in_maps=in_maps, shapes=shapes, nstart=nstart,
                ncnt=ncnt, n_nodes=n_nodes)


def assemble(ctx_run, outs):
    NT = ctx_run['shapes']['NT']
    nstart, ncnt = ctx_run['nstart'], ctx_run['ncnt']
    out = np.zeros((ctx_run['n_nodes'], 40), F32)
    for c in range(NC):
        oc = outs[c]['out']
        for t in range(NT):
            cnt = int(ncnt[c, t])
            if cnt == 0:
                continue
            n0 = int(nstart[c, t])
            out[n0:n0 + cnt] = oc[t * P:t * P + cnt]
    return out


def kernel(x, edge_index, W1, att_src1, att_dst1, b1, W2, att_src2, att_dst2, b2):
    ctx_run = prepare(x, edge_index, W1, att_src1, att_dst1, b1,
                      W2, att_src2, att_dst2, b2)
    res = run_bass_kernel_spmd(ctx_run['nc'], ctx_run['in_maps'],
                               list(range(NC)))
    return assemble(ctx_run, res.results)
